# revision 29
# baseline (speedup 1.0000x reference)
"""Longformer attention TP-sharded Bass kernel for 8 NeuronCores (v3).

Sharding: tensor-parallel over heads. Core d owns heads 2d, 2d+1:
  - Wq/Wk/Wv rows [128d:128(d+1)]  (nn.Linear: q = x @ Wq.T)
  - Wo columns [128d:128(d+1)]
  Each core computes its heads' sparse (windowed+global) attention and a
  full-size out-proj partial; host sums the 8 partials.

v3: fully software-pipelined. Projections (phase A) are interleaved with
attention (phase B): after projection chunk sc (512 tokens), the band
score tiles it unlocks are emitted, strip scores on odd sc, and the
PV/normalize/out-proj for window sc-1. This keeps the PE continuously
busy (p-state stays high) and hides the ACT/DVE elementwise work that
would otherwise serialize phase B.

Per head layout:
  - band tile kb: ONE matmul scoresT [128k, 384q] over the query span
    [128(kb-1), 128(kb+2)); triangular masks post-exp on the outer
    128-col regions (multiplicative, scores are O(1) so no max needed).
  - PV transposed: psum_pv [65, 512q] accumulates lhsT=v_ones[128k, 65]
    x rhs=probsT chunks; row 64 (ones column) = softmax denominator.
  - k=0 global column handled as rank-1 "strips": s(0, q) packed two
    512-query windows per psum tile at partitions {0, 32} (PE quadrant
    anchors), one exp per group, outer-product PV matmuls. The strip is
    the unique start=True initializer of each PV bank (start resets the
    whole bank); the kb0 tile's key-0 row is masked off to compensate.
  - denominators: ACT copies psum row 64 to SBUF (the custom DVE
    reciprocal mis-addresses PSUM at partition base 64), DVE
    reciprocal_approx_fast + bf16 cast, PE outer-product broadcast to
    64 partitions, one DVE multiply -> outT feeds out-proj as lhsT.
  - q=0 global row is patched on the HOST (full-softmax row in numpy
    from bf16-cast inputs; 1 of 4096 rows).
"""

import os
import numpy as np
import ml_dtypes

S = 4096
HIDDEN = 1024
N_CORES = 8
OC = 128          # out-proj contraction dims (head dims) per core = 2 heads x 64
NKB = S // 128    # 32 key blocks
NW = S // 512     # 8 query windows
BF16 = ml_dtypes.bfloat16

_CACHE = {}
LAST_RESULTS = None


def _masks_np():
    """[4, 128, 128]: m_up (f>=p), m_lo (f<=p), m_nr0 (p>0),
    m_lo_nr0 (m_lo & p>0)."""
    p = np.arange(128)[:, None]   # key index within block (partition)
    f = np.arange(128)[None, :]   # query index within block (free)
    m_up = (f >= p)
    m_lo = (f <= p)
    m_nr0 = np.broadcast_to(p > 0, (128, 128))
    return np.stack([m_up, m_lo, m_nr0, m_lo & m_nr0]).astype(BF16)


def _band_clip(kb, w):
    """Overlap of tile kb's query span with window w, or None.
    Returns (psum_col_lo, psum_col_hi, tile_col_lo)."""
    s0 = 128 * (kb - 1)
    s1 = min(128 * (kb + 2), S)
    lo = max(512 * w, s0)
    hi = min(512 * w + 512, s1)
    if lo >= hi:
        return None
    return lo - 512 * w, hi - 512 * w, lo - s0


def _build():
    import concourse.bass as bass
    import concourse.mybir as mybir
    import concourse.tile as tile
    from concourse import bacc

    f32 = mybir.dt.float32
    bf16 = mybir.dt.bfloat16
    Exp = mybir.ActivationFunctionType.Exp

    nc = bacc.Bacc("TRN2", target_bir_lowering=False, debug=False,
                   num_devices=N_CORES)

    xt_d = nc.dram_tensor("xt", [8, 128, 8, 512], bf16, kind="ExternalInput").ap()
    wq_d = nc.dram_tensor("wqt", [128, 8, OC], bf16, kind="ExternalInput").ap()
    wk_d = nc.dram_tensor("wkt", [128, 8, OC], bf16, kind="ExternalInput").ap()
    wv_d = nc.dram_tensor("wvt", [128, 8, OC], bf16, kind="ExternalInput").ap()
    wo_d = nc.dram_tensor("wot", [OC, HIDDEN], bf16, kind="ExternalInput").ap()
    out_d = nc.dram_tensor("partial", [S, HIDDEN], bf16,
                           kind="ExternalOutput").ap()
    mask_d = nc.inline_tensor(_masks_np(), name="masks").ap()
    id_d = nc.inline_tensor(np.eye(128, dtype=BF16), name="ident").ap()

    with tile.TileContext(nc) as tc:
        import contextlib
        with contextlib.ExitStack() as ctx:
            big = ctx.enter_context(tc.tile_pool(name="big", bufs=1))
            probsp = ctx.enter_context(tc.tile_pool(name="probsp", bufs=20))
            tmp = ctx.enter_context(tc.tile_pool(name="tmp", bufs=2))
            outtp = ctx.enter_context(tc.tile_pool(name="outtp", bufs=2))
            stgp = ctx.enter_context(tc.tile_pool(name="stgp", bufs=2))
            psb = ctx.enter_context(tc.tile_pool(name="psb", bufs=3, space="PSUM"))
            psv = ctx.enter_context(tc.tile_pool(name="psv", bufs=1, space="PSUM"))
            pvp = ctx.enter_context(tc.tile_pool(name="pvp", bufs=2, space="PSUM"))
            opp = ctx.enter_context(tc.tile_pool(name="opp", bufs=2, space="PSUM"))

            # ---- resident tensors ----
            xt_sb = big.tile([128, 8, 8, 512], bf16)   # [p, sc, hc, s]
            qt_sb = big.tile([128, S], bf16)          # q.T * 0.125
            kt_sb = big.tile([128, S], bf16)
            v_sb = big.tile([128, NKB, 130], bf16)    # [vA|1|vB|1] per key block
            wq_sb = big.tile([128, 8, OC], bf16)
            wk_sb = big.tile([128, 8, OC], bf16)
            wv_sb = big.tile([128, 8, OC], bf16)
            wo_sb = big.tile([128, HIDDEN], bf16)
            mask_sb = big.tile([128, 4, 128], bf16)
            ones_sb = big.tile([1, 64], bf16)         # bcast lhsT
            id_sb = big.tile([128, 128], bf16)        # PE transpose identity
            vt_p = ctx.enter_context(tc.tile_pool(name="vt_p", bufs=2))
            kts_sb = big.tile([128, 2, 33], bf16)     # strip lhsT: kt0 at col 32j
            v0rep_sb = big.tile([33, 130], bf16)      # v row for key 0, replicated
            # packed strip probs: [group g][head h] -> [33, 512], rows 32j;
            # group g covers windows {2g, 2g+1}
            p0_sb = big.tile([33, 4, 2, 512], bf16)

            # ---- input DMAs: x chunks first (they gate compute), spread
            # across the three DMA-capable queues ----
            dma_engines = [nc.sync, nc.scalar, nc.gpsimd]
            for sc in range(8):
                dma_engines[sc % 3].dma_start(xt_sb[:, sc], xt_d[sc])
            nc.gpsimd.dma_start(wq_sb, wq_d)
            nc.sync.dma_start(wk_sb, wk_d)
            nc.scalar.dma_start(wv_sb, wv_d)
            nc.sync.dma_start(wo_sb, wo_d)
            nc.scalar.dma_start(mask_sb, mask_d.rearrange("m p f -> p m f"))
            nc.vector.memset(v_sb[:, :, 64], 1.0)
            nc.vector.memset(v_sb[:, :, 129], 1.0)
            nc.vector.memset(ones_sb, 1.0)
            nc.vector.memset(kts_sb, 0.0)
            nc.scalar.dma_start(id_sb, id_d)

            probs = {}

            def emit_band(kb, h):
                bp = 64 * h
                pt = probsp.tile([128, 384], bf16, tag="probs", name="probs")
                pss = psb.tile([128, 512], f32, tag="ps512", name="pss")
                if kb == 0:
                    nc.tensor.matmul(pss[:, 0:256], kt_sb[bp:bp + 64, 0:128],
                                     qt_sb[bp:bp + 64, 0:256],
                                     start=True, stop=True)
                    nc.scalar.activation(pt[:, 0:256], pss[:, 0:256], Exp)
                    # {0:128}=m_nr0 (drop key-0 row), {128:256}=m_lo_nr0
                    pr = pt[:, 0:256].rearrange("p (a b) -> p a b", b=128)
                    nc.gpsimd.tensor_mul(pr, pr, mask_sb[:, 2:4, :])
                else:
                    s0 = 128 * (kb - 1)
                    wdt = min(128 * (kb + 2), S) - s0
                    nc.tensor.matmul(pss[:, 0:wdt],
                                     kt_sb[bp:bp + 64, 128 * kb:128 * kb + 128],
                                     qt_sb[bp:bp + 64, s0:s0 + wdt],
                                     start=True, stop=True)
                    nc.scalar.activation(pt[:, 0:wdt], pss[:, 0:wdt], Exp)
                    if kb == NKB - 1:
                        nc.gpsimd.tensor_mul(pt[:, 0:128], pt[:, 0:128],
                                             mask_sb[:, 0, :])
                    else:
                        pr = pt.rearrange("p (a b) -> p a b", b=128)[:, 0::2, :]
                        nc.gpsimd.tensor_mul(pr, pr, mask_sb[:, 0:2, :])
                probs[(kb, h)] = pt

            def emit_strip_group(g):
                # windows 2g, 2g+1 at partition rows 0, 32; needs qt through
                # window 2g+1 -> call at sc = 2g+1
                for h in range(2):
                    bp = 64 * h
                    ps0 = psb.tile([33, 512], f32, tag="ps512", name="ps0")
                    for j in range(2):
                        w = 2 * g + j
                        nc.tensor.matmul(
                            ps0, kts_sb[bp:bp + 64, j, :],
                            qt_sb[bp:bp + 64, 512 * w:512 * w + 512],
                            start=(j == 0), stop=(j == 1))
                    nc.scalar.activation(p0_sb[:, g, h, :], ps0, Exp)

            def emit_outproj(w, outt):
                stg = stgp.tile([128, 4, HIDDEN], bf16, tag="stg", name="stg")
                for c in range(4):
                    for oc in range(2):
                        pso = opp.tile([128, 512], f32, tag="op", name="pso")
                        nc.tensor.matmul(pso, outt[:, 128 * c:128 * c + 128],
                                         wo_sb[:, 512 * oc:512 * oc + 512],
                                         start=True, stop=True)
                        dst = stg[:, c, 512 * oc:512 * oc + 512]
                        if (2 * c + oc) % 3 == 0:
                            nc.scalar.copy(dst, pso)
                        else:
                            nc.vector.tensor_copy(dst, pso)
                dst = out_d[512 * w:512 * w + 512, :].rearrange(
                    "(c p) o -> p c o", p=128)
                nc.sync.dma_start(dst, stg)

            outts = {}

            def emit_pv_finalize(w):
                pvs = []
                for h in range(2):
                    pv = pvp.tile([65, 512], f32, tag="pv", name="pv")
                    g, j = divmod(w, 2)
                    nc.tensor.matmul(
                        pv, v0rep_sb[32 * j:32 * j + 1, 65 * h:65 * h + 65],
                        p0_sb[32 * j:32 * j + 1, g, h, :],
                        start=True, stop=False, skip_group_check=True)
                    if w == 0:
                        nc.tensor.matmul(
                            pv[:, 0:256], v_sb[:, 0, 65 * h:65 * h + 65],
                            probs[(0, h)][:, 0:256],
                            start=False, stop=False, skip_group_check=True)
                    kbs = [kb for kb in range(max(1, 4 * w - 1),
                                              min(4 * w + 5, NKB))
                           if _band_clip(kb, w) is not None]
                    for i, kb in enumerate(kbs):
                        lo, hi, tl = _band_clip(kb, w)
                        nc.tensor.matmul(
                            pv[:, lo:hi], v_sb[:, kb, 65 * h:65 * h + 65],
                            probs[(kb, h)][:, tl:tl + hi - lo],
                            start=False, stop=(i == len(kbs) - 1),
                            skip_group_check=True)
                    pvs.append(pv)
                recips = []
                for h in range(2):
                    rcs = tmp.tile([1, 512], f32, tag="rcs", name="rcs")
                    nc.scalar.copy(rcs, pvs[h][64:65, :])
                    rcf = tmp.tile([1, 512], f32, tag="rcf", name="rcf")
                    nc.vector.reciprocal_approx_fast(rcf, rcs)
                    rcb = tmp.tile([1, 512], bf16, tag="rcb", name="rcb")
                    nc.gpsimd.tensor_copy(rcb, rcf)
                    recips.append(rcb)
                outt = outtp.tile([128, 512], bf16, tag="outt", name="outt")
                for h in range(2):
                    psbc = opp.tile([128, 512], f32, tag="op", name="psbc")
                    nc.tensor.matmul(psbc[0:64, :], ones_sb, recips[h],
                                     start=True, stop=True)
                    bc = tmp.tile([64, 512], bf16, tag="bc", name="bc")
                    if h == 0:
                        nc.scalar.copy(bc, psbc[0:64, :])
                    else:
                        nc.vector.tensor_copy(bc, psbc[0:64, :])
                    nc.vector.tensor_mul(outt[64 * h:64 * h + 64, :],
                                         pvs[h][0:64, :], bc)
                outts[w] = outt

            # ---- fully pipelined main loop ----
            next_kb = 0
            with nc.allow_low_precision("bf16 softmax denominators"):
                for sc in range(8):
                    ssl = slice(sc * 512, (sc + 1) * 512)
                    psq = psb.tile([128, 512], f32, tag="ps512", name="psq")
                    for hc in range(8):
                        nc.tensor.matmul(psq, wq_sb[:, hc, :],
                                         xt_sb[:, sc, hc, :],
                                         start=(hc == 0), stop=(hc == 7))
                    nc.scalar.mul(qt_sb[:, ssl], psq, 0.125)

                    psk = psb.tile([128, 512], f32, tag="ps512", name="psk")
                    for hc in range(8):
                        nc.tensor.matmul(psk, wk_sb[:, hc, :],
                                         xt_sb[:, sc, hc, :],
                                         start=(hc == 0), stop=(hc == 7))
                    nc.vector.tensor_copy(kt_sb[:, ssl], psk)

                    # v transposed (512-free matmuls, ldweights hidden), then
                    # PE-transpose 128-blocks back to natural [token, dim]
                    psvT = psb.tile([128, 512], f32, tag="ps512", name="psvT")
                    for hc in range(8):
                        nc.tensor.matmul(psvT, wv_sb[:, hc, :],
                                         xt_sb[:, sc, hc, :],
                                         start=(hc == 0), stop=(hc == 7))
                    vt_sb = vt_p.tile([128, 512], bf16, tag="vt", name="vt")
                    nc.vector.tensor_copy(vt_sb, psvT)
                    for b in range(4):
                        kb = sc * 4 + b
                        psvt = psv.tile([128, 256], bf16, tag="psv",
                                        name="psvt")
                        nc.tensor.transpose(
                            psvt[:, 0:128], vt_sb[:, b * 128:b * 128 + 128],
                            id_sb)
                        vdst = v_sb[:, kb, :].rearrange("p (h c) -> p h c", h=2)
                        nc.vector.tensor_copy(
                            vdst[:, :, 0:64],
                            psvt[:, 0:128].rearrange("p (h c) -> p h c", h=2))

                    if sc == 0:
                        for j in range(2):
                            nc.vector.tensor_copy(
                                kts_sb[:, j, 32 * j:32 * j + 1], kt_sb[:, 0:1])
                        nc.gpsimd.partition_broadcast(v0rep_sb, v_sb[0:1, 0, :])

                    # band tiles unlocked by this projection chunk
                    hi_kb = min(4 * sc + 2, NKB - 2) if sc < 7 else NKB - 1
                    for h in range(2):
                        for kb in range(next_kb, hi_kb + 1):
                            emit_band(kb, h)
                    next_kb = hi_kb + 1

                    if sc % 2 == 1:
                        emit_strip_group(sc // 2)

                    if sc >= 1:
                        emit_pv_finalize(sc - 1)
                        if sc >= 2:
                            emit_outproj(sc - 2, outts.pop(sc - 2))
                # drain: last window + trailing out-projs
                emit_pv_finalize(NW - 1)
                emit_outproj(NW - 2, outts.pop(NW - 2))
                emit_outproj(NW - 1, outts.pop(NW - 1))

    nc.compile()
    return nc


def _host_row0(x, Wq, Wk, Wv, Wo):
    """Full-softmax attention output row for global query 0 (all 16 heads)."""
    f32 = np.float32
    xb = np.asarray(x)[0].astype(BF16)
    q0 = ((xb[0:1].astype(f32) @ np.asarray(Wq).astype(BF16).astype(f32).T)
          * f32(0.125)).astype(BF16).astype(f32)[0]
    K = (xb.astype(f32) @ np.asarray(Wk).astype(BF16).astype(f32).T
         ).astype(BF16).astype(f32)
    V = (xb.astype(f32) @ np.asarray(Wv).astype(BF16).astype(f32).T
         ).astype(BF16).astype(f32)
    out0 = np.empty(HIDDEN, f32)
    for h in range(16):
        sl = slice(64 * h, 64 * h + 64)
        s = K[:, sl] @ q0[sl]
        p = np.exp(s - s.max())
        out0[sl] = (p @ V[:, sl]) / p.sum()
    out0 = out0.astype(BF16).astype(f32)
    return out0 @ np.asarray(Wo).astype(BF16).astype(f32).T


def kernel(x, Wq, Wk, Wv, Wo):
    from concourse import bass_utils

    x = np.asarray(x)
    B = x.shape[0]
    xb = x[0].astype(BF16)
    xt = np.ascontiguousarray(
        xb.reshape(8, 512, 8, 128).transpose(0, 3, 2, 1))

    def wprep(W, rs):
        wt = np.asarray(W)[rs, :].T.astype(BF16)
        return np.ascontiguousarray(
            wt.reshape(8, 128, OC).transpose(1, 0, 2))

    in_maps = []
    for d in range(N_CORES):
        rs = slice(OC * d, OC * (d + 1))
        in_maps.append({
            "xt": xt,
            "wqt": wprep(Wq, rs),
            "wkt": wprep(Wk, rs),
            "wvt": wprep(Wv, rs),
            "wot": np.ascontiguousarray(np.asarray(Wo)[:, rs].T.astype(BF16)),
        })

    if "nc" not in _CACHE:
        _CACHE["nc"] = _build()
    nc = _CACHE["nc"]

    res = bass_utils.run_bass_kernel_spmd(
        nc, in_maps, core_ids=list(range(N_CORES)),
        trace=bool(os.environ.get("KERNEL_TRACE")))
    global LAST_RESULTS
    LAST_RESULTS = res

    out = np.zeros((S, HIDDEN), np.float64)
    for r in res.results:
        out += r["partial"].astype(np.float64)
    out[0, :] = _host_row0(x, Wq, Wk, Wv, Wo)
    return out.reshape(B, S, HIDDEN).astype(np.float32)


# revision 30
# speedup vs baseline: 1.1406x; 1.1406x over previous
"""Longformer attention TP-sharded Bass kernel for 8 NeuronCores (v3).

Sharding: tensor-parallel over heads. Core d owns heads 2d, 2d+1:
  - Wq/Wk/Wv rows [128d:128(d+1)]  (nn.Linear: q = x @ Wq.T)
  - Wo columns [128d:128(d+1)]
  Each core computes its heads' sparse (windowed+global) attention and a
  full-size out-proj partial; host sums the 8 partials.

v3: fully software-pipelined. Projections (phase A) are interleaved with
attention (phase B): after projection chunk sc (512 tokens), the band
score tiles it unlocks are emitted, strip scores on odd sc, and the
PV/normalize/out-proj for window sc-1. This keeps the PE continuously
busy (p-state stays high) and hides the ACT/DVE elementwise work that
would otherwise serialize phase B.

Per head layout:
  - band tile kb: ONE matmul scoresT [128k, 384q] over the query span
    [128(kb-1), 128(kb+2)); triangular masks post-exp on the outer
    128-col regions (multiplicative, scores are O(1) so no max needed).
  - PV transposed: psum_pv [65, 512q] accumulates lhsT=v_ones[128k, 65]
    x rhs=probsT chunks; row 64 (ones column) = softmax denominator.
  - k=0 global column handled as rank-1 "strips": s(0, q) packed two
    512-query windows per psum tile at partitions {0, 32} (PE quadrant
    anchors), one exp per group, outer-product PV matmuls. The strip is
    the unique start=True initializer of each PV bank (start resets the
    whole bank); the kb0 tile's key-0 row is masked off to compensate.
  - denominators: ACT copies psum row 64 to SBUF (the custom DVE
    reciprocal mis-addresses PSUM at partition base 64), DVE
    reciprocal_approx_fast + bf16 cast, PE outer-product broadcast to
    64 partitions, one DVE multiply -> outT feeds out-proj as lhsT.
  - q=0 global row is patched on the HOST (full-softmax row in numpy
    from bf16-cast inputs; 1 of 4096 rows).
"""

import os
import numpy as np
import ml_dtypes

S = 4096
HIDDEN = 1024
N_CORES = 8
OC = 128          # out-proj contraction dims (head dims) per core = 2 heads x 64
NKB = S // 128    # 32 key blocks
NW = S // 512     # 8 query windows
BF16 = ml_dtypes.bfloat16

_CACHE = {}
LAST_RESULTS = None


def _masks_np():
    """[4, 128, 128]: m_up (f>=p), m_lo (f<=p), m_nr0 (p>0),
    m_lo_nr0 (m_lo & p>0)."""
    p = np.arange(128)[:, None]   # key index within block (partition)
    f = np.arange(128)[None, :]   # query index within block (free)
    m_up = (f >= p)
    m_lo = (f <= p)
    m_nr0 = np.broadcast_to(p > 0, (128, 128))
    return np.stack([m_up, m_lo, m_nr0, m_lo & m_nr0]).astype(BF16)


def _band_clip(kb, w):
    """Overlap of tile kb's query span with window w, or None.
    Returns (psum_col_lo, psum_col_hi, tile_col_lo)."""
    s0 = 128 * (kb - 1)
    s1 = min(128 * (kb + 2), S)
    lo = max(512 * w, s0)
    hi = min(512 * w + 512, s1)
    if lo >= hi:
        return None
    return lo - 512 * w, hi - 512 * w, lo - s0


def _build():
    import concourse.bass as bass
    import concourse.mybir as mybir
    import concourse.tile as tile
    from concourse import bacc

    f32 = mybir.dt.float32
    bf16 = mybir.dt.bfloat16
    Exp = mybir.ActivationFunctionType.Exp

    nc = bacc.Bacc("TRN2", target_bir_lowering=False, debug=False,
                   num_devices=N_CORES)

    xt_d = nc.dram_tensor("xt", [8, 128, 8, 512], bf16, kind="ExternalInput").ap()
    wq_d = nc.dram_tensor("wqt", [128, 8, OC], bf16, kind="ExternalInput").ap()
    wk_d = nc.dram_tensor("wkt", [128, 8, OC], bf16, kind="ExternalInput").ap()
    wv_d = nc.dram_tensor("wvt", [128, 8, OC], bf16, kind="ExternalInput").ap()
    wo_d = nc.dram_tensor("wot", [OC, HIDDEN], bf16, kind="ExternalInput").ap()
    out_d = nc.dram_tensor("partial", [S, HIDDEN], bf16,
                           kind="ExternalOutput").ap()
    mask_d = nc.inline_tensor(_masks_np(), name="masks").ap()
    id_d = nc.inline_tensor(np.eye(128, dtype=BF16), name="ident").ap()

    with tile.TileContext(nc) as tc:
        import contextlib
        with contextlib.ExitStack() as ctx:
            big = ctx.enter_context(tc.tile_pool(name="big", bufs=1))
            probsp = ctx.enter_context(tc.tile_pool(name="probsp", bufs=20))
            tmp = ctx.enter_context(tc.tile_pool(name="tmp", bufs=2))
            outtp = ctx.enter_context(tc.tile_pool(name="outtp", bufs=2))
            stgp = ctx.enter_context(tc.tile_pool(name="stgp", bufs=2))
            psb = ctx.enter_context(tc.tile_pool(name="psb", bufs=3, space="PSUM"))
            psv = ctx.enter_context(tc.tile_pool(name="psv", bufs=1, space="PSUM"))
            pvp = ctx.enter_context(tc.tile_pool(name="pvp", bufs=2, space="PSUM"))
            opp = ctx.enter_context(tc.tile_pool(name="opp", bufs=2, space="PSUM"))

            # ---- resident tensors ----
            xt_sb = big.tile([128, 8, 8, 512], bf16)   # [p, sc, hc, s]
            qt_sb = big.tile([128, S], bf16)          # q.T * 0.125
            kt_sb = big.tile([128, S], bf16)
            v_sb = big.tile([128, NKB, 130], bf16)    # [vA|1|vB|1] per key block
            wq_sb = big.tile([128, 8, OC], bf16)
            wk_sb = big.tile([128, 8, OC], bf16)
            wv_sb = big.tile([128, 8, OC], bf16)
            wo_sb = big.tile([128, HIDDEN], bf16)
            mask_sb = big.tile([128, 4, 128], bf16)
            ones_sb = big.tile([1, 64], bf16)         # bcast lhsT
            id_sb = big.tile([128, 128], bf16)        # PE transpose identity
            vt_p = ctx.enter_context(tc.tile_pool(name="vt_p", bufs=2))
            kts_sb = big.tile([128, 2, 33], bf16)     # strip lhsT: kt0 at col 32j
            v0rep_sb = big.tile([33, 130], bf16)      # v row for key 0, replicated
            # packed strip probs: [group g][head h] -> [33, 512], rows 32j;
            # group g covers windows {2g, 2g+1}
            p0_sb = big.tile([33, 4, 2, 512], bf16)

            # ---- input DMAs: x chunks first (they gate compute), spread
            # across the three DMA-capable queues ----
            dma_engines = [nc.sync, nc.scalar, nc.gpsimd]
            for sc in range(8):
                dma_engines[sc % 3].dma_start(xt_sb[:, sc], xt_d[sc])
            nc.gpsimd.dma_start(wq_sb, wq_d)
            nc.sync.dma_start(wk_sb, wk_d)
            nc.scalar.dma_start(wv_sb, wv_d)
            nc.sync.dma_start(wo_sb, wo_d)
            nc.scalar.dma_start(mask_sb, mask_d.rearrange("m p f -> p m f"))
            nc.vector.memset(v_sb[:, :, 64], 1.0)
            nc.vector.memset(v_sb[:, :, 129], 1.0)
            nc.vector.memset(ones_sb, 1.0)
            nc.vector.memset(kts_sb, 0.0)
            nc.scalar.dma_start(id_sb, id_d)

            probs = {}

            def emit_band(kb, h):
                bp = 64 * h
                pt = probsp.tile([128, 384], bf16, tag="probs", name="probs")
                pss = psb.tile([128, 512], f32, tag="ps512", name="pss")
                if kb == 0:
                    nc.tensor.matmul(pss[:, 0:256], kt_sb[bp:bp + 64, 0:128],
                                     qt_sb[bp:bp + 64, 0:256],
                                     start=True, stop=True)
                    nc.scalar.activation(pt[:, 0:256], pss[:, 0:256], Exp)
                    # {0:128}=m_nr0 (drop key-0 row), {128:256}=m_lo_nr0
                    pr = pt[:, 0:256].rearrange("p (a b) -> p a b", b=128)
                    nc.gpsimd.tensor_mul(pr, pr, mask_sb[:, 2:4, :])
                else:
                    s0 = 128 * (kb - 1)
                    wdt = min(128 * (kb + 2), S) - s0
                    nc.tensor.matmul(pss[:, 0:wdt],
                                     kt_sb[bp:bp + 64, 128 * kb:128 * kb + 128],
                                     qt_sb[bp:bp + 64, s0:s0 + wdt],
                                     start=True, stop=True)
                    nc.scalar.activation(pt[:, 0:wdt], pss[:, 0:wdt], Exp)
                    if kb == NKB - 1:
                        nc.gpsimd.tensor_mul(pt[:, 0:128], pt[:, 0:128],
                                             mask_sb[:, 0, :])
                    else:
                        pr = pt.rearrange("p (a b) -> p a b", b=128)[:, 0::2, :]
                        nc.gpsimd.tensor_mul(pr, pr, mask_sb[:, 0:2, :])
                probs[(kb, h)] = pt

            def emit_strip_group(g):
                # windows 2g, 2g+1 at partition rows 0, 32; needs qt through
                # window 2g+1 -> call at sc = 2g+1
                for h in range(2):
                    bp = 64 * h
                    ps0 = psb.tile([33, 512], f32, tag="ps512", name="ps0")
                    for j in range(2):
                        w = 2 * g + j
                        nc.tensor.matmul(
                            ps0, kts_sb[bp:bp + 64, j, :],
                            qt_sb[bp:bp + 64, 512 * w:512 * w + 512],
                            start=(j == 0), stop=(j == 1))
                    nc.scalar.activation(p0_sb[:, g, h, :], ps0, Exp)

            def emit_outproj(w, outt):
                stg = stgp.tile([128, 4, HIDDEN], bf16, tag="stg", name="stg")
                for c in range(4):
                    for oc in range(2):
                        pso = opp.tile([128, 512], f32, tag="op", name="pso")
                        nc.tensor.matmul(pso, outt[:, 128 * c:128 * c + 128],
                                         wo_sb[:, 512 * oc:512 * oc + 512],
                                         start=True, stop=True)
                        dst = stg[:, c, 512 * oc:512 * oc + 512]
                        if (2 * c + oc) % 3 == 0:
                            nc.scalar.copy(dst, pso)
                        else:
                            nc.vector.tensor_copy(dst, pso)
                dst = out_d[512 * w:512 * w + 512, :].rearrange(
                    "(c p) o -> p c o", p=128)
                nc.sync.dma_start(dst, stg)

            outts = {}

            def emit_pv_finalize(w):
                pvs = []
                for h in range(2):
                    pv = pvp.tile([65, 512], f32, tag="pv", name="pv")
                    g, j = divmod(w, 2)
                    nc.tensor.matmul(
                        pv, v0rep_sb[32 * j:32 * j + 1, 65 * h:65 * h + 65],
                        p0_sb[32 * j:32 * j + 1, g, h, :],
                        start=True, stop=False, skip_group_check=True)
                    if w == 0:
                        nc.tensor.matmul(
                            pv[:, 0:256], v_sb[:, 0, 65 * h:65 * h + 65],
                            probs[(0, h)][:, 0:256],
                            start=False, stop=False, skip_group_check=True)
                    kbs = [kb for kb in range(max(1, 4 * w - 1),
                                              min(4 * w + 5, NKB))
                           if _band_clip(kb, w) is not None]
                    for i, kb in enumerate(kbs):
                        lo, hi, tl = _band_clip(kb, w)
                        nc.tensor.matmul(
                            pv[:, lo:hi], v_sb[:, kb, 65 * h:65 * h + 65],
                            probs[(kb, h)][:, tl:tl + hi - lo],
                            start=False, stop=(i == len(kbs) - 1),
                            skip_group_check=True)
                    pvs.append(pv)
                recips = []
                for h in range(2):
                    rcs = tmp.tile([1, 512], f32, tag="rcs", name="rcs")
                    nc.scalar.copy(rcs, pvs[h][64:65, :])
                    rcf = tmp.tile([1, 512], f32, tag="rcf", name="rcf")
                    nc.vector.reciprocal_approx_fast(rcf, rcs)
                    rcb = tmp.tile([1, 512], bf16, tag="rcb", name="rcb")
                    nc.vector.tensor_copy(rcb, rcf)
                    recips.append(rcb)
                outt = outtp.tile([128, 512], bf16, tag="outt", name="outt")
                for h in range(2):
                    psbc = opp.tile([128, 512], f32, tag="op", name="psbc")
                    nc.tensor.matmul(psbc[0:64, :], ones_sb, recips[h],
                                     start=True, stop=True)
                    bc = tmp.tile([64, 512], bf16, tag="bc", name="bc")
                    if h == 0:
                        nc.scalar.copy(bc, psbc[0:64, :])
                    else:
                        nc.vector.tensor_copy(bc, psbc[0:64, :])
                    nc.vector.tensor_mul(outt[64 * h:64 * h + 64, :],
                                         pvs[h][0:64, :], bc)
                outts[w] = outt

            # ---- fully pipelined main loop ----
            next_kb = 0
            with nc.allow_low_precision("bf16 softmax denominators"):
                for sc in range(8):
                    ssl = slice(sc * 512, (sc + 1) * 512)
                    psq = psb.tile([128, 512], f32, tag="ps512", name="psq")
                    for hc in range(8):
                        nc.tensor.matmul(psq, wq_sb[:, hc, :],
                                         xt_sb[:, sc, hc, :],
                                         start=(hc == 0), stop=(hc == 7))
                    nc.scalar.mul(qt_sb[:, ssl], psq, 0.125)

                    psk = psb.tile([128, 512], f32, tag="ps512", name="psk")
                    for hc in range(8):
                        nc.tensor.matmul(psk, wk_sb[:, hc, :],
                                         xt_sb[:, sc, hc, :],
                                         start=(hc == 0), stop=(hc == 7))
                    nc.vector.tensor_copy(kt_sb[:, ssl], psk)

                    # v transposed (512-free matmuls, ldweights hidden), then
                    # PE-transpose 128-blocks back to natural [token, dim]
                    psvT = psb.tile([128, 512], f32, tag="ps512", name="psvT")
                    for hc in range(8):
                        nc.tensor.matmul(psvT, wv_sb[:, hc, :],
                                         xt_sb[:, sc, hc, :],
                                         start=(hc == 0), stop=(hc == 7))
                    vt_sb = vt_p.tile([128, 512], bf16, tag="vt", name="vt")
                    nc.vector.tensor_copy(vt_sb, psvT)
                    for b in range(4):
                        kb = sc * 4 + b
                        psvt = psv.tile([128, 256], bf16, tag="psv",
                                        name="psvt")
                        nc.tensor.transpose(
                            psvt[:, 0:128], vt_sb[:, b * 128:b * 128 + 128],
                            id_sb)
                        vdst = v_sb[:, kb, :].rearrange("p (h c) -> p h c", h=2)
                        nc.vector.tensor_copy(
                            vdst[:, :, 0:64],
                            psvt[:, 0:128].rearrange("p (h c) -> p h c", h=2))

                    if sc == 0:
                        for j in range(2):
                            nc.vector.tensor_copy(
                                kts_sb[:, j, 32 * j:32 * j + 1], kt_sb[:, 0:1])
                        nc.gpsimd.partition_broadcast(v0rep_sb, v_sb[0:1, 0, :])

                    # band tiles unlocked by this projection chunk
                    hi_kb = min(4 * sc + 2, NKB - 2) if sc < 7 else NKB - 1
                    for h in range(2):
                        for kb in range(next_kb, hi_kb + 1):
                            emit_band(kb, h)
                    next_kb = hi_kb + 1

                    if sc % 2 == 1:
                        emit_strip_group(sc // 2)

                    if sc >= 1:
                        emit_pv_finalize(sc - 1)
                        if sc >= 2:
                            emit_outproj(sc - 2, outts.pop(sc - 2))
                # drain: last window + trailing out-projs
                emit_pv_finalize(NW - 1)
                emit_outproj(NW - 2, outts.pop(NW - 2))
                emit_outproj(NW - 1, outts.pop(NW - 1))

    nc.compile()
    return nc


def _host_row0(x, Wq, Wk, Wv, Wo):
    """Full-softmax attention output row for global query 0 (all 16 heads)."""
    f32 = np.float32
    xb = np.asarray(x)[0].astype(BF16)
    q0 = ((xb[0:1].astype(f32) @ np.asarray(Wq).astype(BF16).astype(f32).T)
          * f32(0.125)).astype(BF16).astype(f32)[0]
    K = (xb.astype(f32) @ np.asarray(Wk).astype(BF16).astype(f32).T
         ).astype(BF16).astype(f32)
    V = (xb.astype(f32) @ np.asarray(Wv).astype(BF16).astype(f32).T
         ).astype(BF16).astype(f32)
    out0 = np.empty(HIDDEN, f32)
    for h in range(16):
        sl = slice(64 * h, 64 * h + 64)
        s = K[:, sl] @ q0[sl]
        p = np.exp(s - s.max())
        out0[sl] = (p @ V[:, sl]) / p.sum()
    out0 = out0.astype(BF16).astype(f32)
    return out0 @ np.asarray(Wo).astype(BF16).astype(f32).T


def kernel(x, Wq, Wk, Wv, Wo):
    from concourse import bass_utils

    x = np.asarray(x)
    B = x.shape[0]
    xb = x[0].astype(BF16)
    xt = np.ascontiguousarray(
        xb.reshape(8, 512, 8, 128).transpose(0, 3, 2, 1))

    def wprep(W, rs):
        wt = np.asarray(W)[rs, :].T.astype(BF16)
        return np.ascontiguousarray(
            wt.reshape(8, 128, OC).transpose(1, 0, 2))

    in_maps = []
    for d in range(N_CORES):
        rs = slice(OC * d, OC * (d + 1))
        in_maps.append({
            "xt": xt,
            "wqt": wprep(Wq, rs),
            "wkt": wprep(Wk, rs),
            "wvt": wprep(Wv, rs),
            "wot": np.ascontiguousarray(np.asarray(Wo)[:, rs].T.astype(BF16)),
        })

    if "nc" not in _CACHE:
        _CACHE["nc"] = _build()
    nc = _CACHE["nc"]

    res = bass_utils.run_bass_kernel_spmd(
        nc, in_maps, core_ids=list(range(N_CORES)),
        trace=bool(os.environ.get("KERNEL_TRACE")))
    global LAST_RESULTS
    LAST_RESULTS = res

    out = np.zeros((S, HIDDEN), np.float64)
    for r in res.results:
        out += r["partial"].astype(np.float64)
    out[0, :] = _host_row0(x, Wq, Wk, Wv, Wo)
    return out.reshape(B, S, HIDDEN).astype(np.float32)


# revision 32
# speedup vs baseline: 1.1871x; 1.0408x over previous
"""Longformer attention TP-sharded Bass kernel for 8 NeuronCores (v3).

Sharding: tensor-parallel over heads. Core d owns heads 2d, 2d+1:
  - Wq/Wk/Wv rows [128d:128(d+1)]  (nn.Linear: q = x @ Wq.T)
  - Wo columns [128d:128(d+1)]
  Each core computes its heads' sparse (windowed+global) attention and a
  full-size out-proj partial; host sums the 8 partials.

v3: fully software-pipelined. Projections (phase A) are interleaved with
attention (phase B): after projection chunk sc (512 tokens), the band
score tiles it unlocks are emitted, strip scores on odd sc, and the
PV/normalize/out-proj for window sc-1. This keeps the PE continuously
busy (p-state stays high) and hides the ACT/DVE elementwise work that
would otherwise serialize phase B.

Per head layout:
  - band tile kb: ONE matmul scoresT [128k, 384q] over the query span
    [128(kb-1), 128(kb+2)); triangular masks post-exp on the outer
    128-col regions (multiplicative, scores are O(1) so no max needed).
  - PV transposed: psum_pv [65, 512q] accumulates lhsT=v_ones[128k, 65]
    x rhs=probsT chunks; row 64 (ones column) = softmax denominator.
  - k=0 global column handled as rank-1 "strips": s(0, q) packed two
    512-query windows per psum tile at partitions {0, 32} (PE quadrant
    anchors), one exp per group, outer-product PV matmuls. The strip is
    the unique start=True initializer of each PV bank (start resets the
    whole bank); the kb0 tile's key-0 row is masked off to compensate.
  - denominators: ACT copies psum row 64 to SBUF (the custom DVE
    reciprocal mis-addresses PSUM at partition base 64), DVE
    reciprocal_approx_fast + bf16 cast, PE outer-product broadcast to
    64 partitions, one DVE multiply -> outT feeds out-proj as lhsT.
  - q=0 global row is patched on the HOST (full-softmax row in numpy
    from bf16-cast inputs; 1 of 4096 rows).
"""

import os
import numpy as np
import ml_dtypes

S = 4096
HIDDEN = 1024
N_CORES = 8
OC = 128          # out-proj contraction dims (head dims) per core = 2 heads x 64
NKB = S // 128    # 32 key blocks
NW = S // 512     # 8 query windows
BF16 = ml_dtypes.bfloat16

_CACHE = {}
LAST_RESULTS = None


def _masks_np():
    """[4, 128, 128]: m_up (f>=p), m_lo (f<=p), m_nr0 (p>0),
    m_lo_nr0 (m_lo & p>0)."""
    p = np.arange(128)[:, None]   # key index within block (partition)
    f = np.arange(128)[None, :]   # query index within block (free)
    m_up = (f >= p)
    m_lo = (f <= p)
    m_nr0 = np.broadcast_to(p > 0, (128, 128))
    return np.stack([m_up, m_lo, m_nr0, m_lo & m_nr0]).astype(BF16)


def _band_clip(kb, w):
    """Overlap of tile kb's query span with window w, or None.
    Returns (psum_col_lo, psum_col_hi, tile_col_lo)."""
    s0 = 128 * (kb - 1)
    s1 = min(128 * (kb + 2), S)
    lo = max(512 * w, s0)
    hi = min(512 * w + 512, s1)
    if lo >= hi:
        return None
    return lo - 512 * w, hi - 512 * w, lo - s0


def _build():
    import concourse.bass as bass
    import concourse.mybir as mybir
    import concourse.tile as tile
    from concourse import bacc

    f32 = mybir.dt.float32
    bf16 = mybir.dt.bfloat16
    Exp = mybir.ActivationFunctionType.Exp

    nc = bacc.Bacc("TRN2", target_bir_lowering=False, debug=False,
                   num_devices=N_CORES)

    xt_d = nc.dram_tensor("xt", [8, 128, 8, 512], bf16, kind="ExternalInput").ap()
    wq_d = nc.dram_tensor("wqt", [128, 8, OC], bf16, kind="ExternalInput").ap()
    wk_d = nc.dram_tensor("wkt", [128, 8, OC], bf16, kind="ExternalInput").ap()
    wv_d = nc.dram_tensor("wvt", [128, 8, OC], bf16, kind="ExternalInput").ap()
    wo_d = nc.dram_tensor("wot", [OC, HIDDEN], bf16, kind="ExternalInput").ap()
    out_d = nc.dram_tensor("partial", [S, HIDDEN], bf16,
                           kind="ExternalOutput").ap()
    mask_d = nc.inline_tensor(_masks_np(), name="masks").ap()
    id_d = nc.inline_tensor(np.eye(128, dtype=BF16), name="ident").ap()

    with tile.TileContext(nc) as tc:
        import contextlib
        with contextlib.ExitStack() as ctx:
            big = ctx.enter_context(tc.tile_pool(name="big", bufs=1))
            probsp = ctx.enter_context(tc.tile_pool(name="probsp", bufs=20))
            tmp = ctx.enter_context(tc.tile_pool(name="tmp", bufs=2))
            outtp = ctx.enter_context(tc.tile_pool(name="outtp", bufs=2))
            stgp = ctx.enter_context(tc.tile_pool(name="stgp", bufs=2))
            psb = ctx.enter_context(tc.tile_pool(name="psb", bufs=3, space="PSUM"))
            psv = ctx.enter_context(tc.tile_pool(name="psv", bufs=1, space="PSUM"))
            pvp = ctx.enter_context(tc.tile_pool(name="pvp", bufs=2, space="PSUM"))
            opp = ctx.enter_context(tc.tile_pool(name="opp", bufs=2, space="PSUM"))

            # ---- resident tensors ----
            xt_sb = big.tile([128, 8, 8, 512], bf16)   # [p, sc, hc, s]
            qt_sb = big.tile([128, S], bf16)          # q.T * 0.125
            kt_sb = big.tile([128, S], bf16)
            v_sb = big.tile([128, NKB, 130], bf16)    # [vA|1|vB|1] per key block
            wq_sb = big.tile([128, 8, OC], bf16)
            wk_sb = big.tile([128, 8, OC], bf16)
            wv_sb = big.tile([128, 8, OC], bf16)
            wo_sb = big.tile([128, HIDDEN], bf16)
            mask_sb = big.tile([128, 4, 128], bf16)
            ones_sb = big.tile([1, 64], bf16)         # bcast lhsT
            id_sb = big.tile([128, 128], bf16)        # PE transpose identity
            vt_p = ctx.enter_context(tc.tile_pool(name="vt_p", bufs=2))
            kts_sb = big.tile([128, 2, 33], bf16)     # strip lhsT: kt0 at col 32j
            v0rep_sb = big.tile([33, 130], bf16)      # v row for key 0, replicated
            # packed strip probs: [group g][head h] -> [33, 512], rows 32j;
            # group g covers windows {2g, 2g+1}
            p0_sb = big.tile([33, 4, 2, 512], bf16)

            # ---- input DMAs: only the first two x chunks up front so chunk
            # 0 gets the full DMA bandwidth; the rest prefetch inside the
            # loop two iterations ahead ----
            nc.sync.dma_start(xt_sb[:, 0], xt_d[0])
            nc.sync.dma_start(xt_sb[:, 1], xt_d[1])
            nc.gpsimd.dma_start(wq_sb, wq_d)
            nc.sync.dma_start(wk_sb, wk_d)
            nc.scalar.dma_start(wv_sb, wv_d)
            nc.sync.dma_start(wo_sb, wo_d)
            nc.scalar.dma_start(mask_sb, mask_d.rearrange("m p f -> p m f"))
            nc.vector.memset(v_sb[:, :, 64], 1.0)
            nc.vector.memset(v_sb[:, :, 129], 1.0)
            nc.vector.memset(ones_sb, 1.0)
            nc.vector.memset(kts_sb, 0.0)
            nc.scalar.dma_start(id_sb, id_d)

            probs = {}

            def emit_band(kb, h):
                bp = 64 * h
                pt = probsp.tile([128, 384], bf16, tag="probs", name="probs")
                pss = psb.tile([128, 512], f32, tag="ps512", name="pss")
                if kb == 0:
                    nc.tensor.matmul(pss[:, 0:256], kt_sb[bp:bp + 64, 0:128],
                                     qt_sb[bp:bp + 64, 0:256],
                                     start=True, stop=True)
                    nc.scalar.activation(pt[:, 0:256], pss[:, 0:256], Exp)
                    # {0:128}=m_nr0 (drop key-0 row), {128:256}=m_lo_nr0
                    pr = pt[:, 0:256].rearrange("p (a b) -> p a b", b=128)
                    nc.gpsimd.tensor_mul(pr, pr, mask_sb[:, 2:4, :])
                else:
                    s0 = 128 * (kb - 1)
                    wdt = min(128 * (kb + 2), S) - s0
                    nc.tensor.matmul(pss[:, 0:wdt],
                                     kt_sb[bp:bp + 64, 128 * kb:128 * kb + 128],
                                     qt_sb[bp:bp + 64, s0:s0 + wdt],
                                     start=True, stop=True)
                    nc.scalar.activation(pt[:, 0:wdt], pss[:, 0:wdt], Exp)
                    if kb == NKB - 1:
                        nc.gpsimd.tensor_mul(pt[:, 0:128], pt[:, 0:128],
                                             mask_sb[:, 0, :])
                    else:
                        pr = pt.rearrange("p (a b) -> p a b", b=128)[:, 0::2, :]
                        nc.gpsimd.tensor_mul(pr, pr, mask_sb[:, 0:2, :])
                probs[(kb, h)] = pt

            def emit_strip_group(g):
                # windows 2g, 2g+1 at partition rows 0, 32; needs qt through
                # window 2g+1 -> call at sc = 2g+1
                for h in range(2):
                    bp = 64 * h
                    ps0 = psb.tile([33, 512], f32, tag="ps512", name="ps0")
                    for j in range(2):
                        w = 2 * g + j
                        nc.tensor.matmul(
                            ps0, kts_sb[bp:bp + 64, j, :],
                            qt_sb[bp:bp + 64, 512 * w:512 * w + 512],
                            start=(j == 0), stop=(j == 1))
                    nc.scalar.activation(p0_sb[:, g, h, :], ps0, Exp)

            def emit_outproj(w, outt):
                stg = stgp.tile([128, 4, HIDDEN], bf16, tag="stg", name="stg")
                for c in range(4):
                    for oc in range(2):
                        pso = opp.tile([128, 512], f32, tag="op", name="pso")
                        nc.tensor.matmul(pso, outt[:, 128 * c:128 * c + 128],
                                         wo_sb[:, 512 * oc:512 * oc + 512],
                                         start=True, stop=True)
                        dst = stg[:, c, 512 * oc:512 * oc + 512]
                        if (2 * c + oc) % 3 == 0:
                            nc.scalar.copy(dst, pso)
                        else:
                            nc.vector.tensor_copy(dst, pso)
                dst = out_d[512 * w:512 * w + 512, :].rearrange(
                    "(c p) o -> p c o", p=128)
                nc.sync.dma_start(dst, stg)

            outts = {}

            def emit_pv_finalize(w):
                pvs = []
                for h in range(2):
                    pv = pvp.tile([65, 512], f32, tag="pv", name="pv")
                    g, j = divmod(w, 2)
                    nc.tensor.matmul(
                        pv, v0rep_sb[32 * j:32 * j + 1, 65 * h:65 * h + 65],
                        p0_sb[32 * j:32 * j + 1, g, h, :],
                        start=True, stop=False, skip_group_check=True)
                    if w == 0:
                        nc.tensor.matmul(
                            pv[:, 0:256], v_sb[:, 0, 65 * h:65 * h + 65],
                            probs[(0, h)][:, 0:256],
                            start=False, stop=False, skip_group_check=True)
                    kbs = [kb for kb in range(max(1, 4 * w - 1),
                                              min(4 * w + 5, NKB))
                           if _band_clip(kb, w) is not None]
                    for i, kb in enumerate(kbs):
                        lo, hi, tl = _band_clip(kb, w)
                        nc.tensor.matmul(
                            pv[:, lo:hi], v_sb[:, kb, 65 * h:65 * h + 65],
                            probs[(kb, h)][:, tl:tl + hi - lo],
                            start=False, stop=(i == len(kbs) - 1),
                            skip_group_check=True)
                    pvs.append(pv)
                recips = []
                for h in range(2):
                    rcs = tmp.tile([1, 512], f32, tag="rcs", name="rcs")
                    nc.scalar.copy(rcs, pvs[h][64:65, :])
                    rcf = tmp.tile([1, 512], f32, tag="rcf", name="rcf")
                    nc.vector.reciprocal_approx_fast(rcf, rcs)
                    rcb = tmp.tile([1, 512], bf16, tag="rcb", name="rcb")
                    nc.vector.tensor_copy(rcb, rcf)
                    recips.append(rcb)
                outt = outtp.tile([128, 512], bf16, tag="outt", name="outt")
                for h in range(2):
                    psbc = opp.tile([128, 512], f32, tag="op", name="psbc")
                    nc.tensor.matmul(psbc[0:64, :], ones_sb, recips[h],
                                     start=True, stop=True)
                    bc = tmp.tile([64, 512], bf16, tag="bc", name="bc")
                    if h == 0:
                        nc.scalar.copy(bc, psbc[0:64, :])
                    else:
                        nc.vector.tensor_copy(bc, psbc[0:64, :])
                    nc.vector.tensor_mul(outt[64 * h:64 * h + 64, :],
                                         pvs[h][0:64, :], bc)
                outts[w] = outt

            # ---- fully pipelined main loop ----
            next_kb = 0
            with nc.allow_low_precision("bf16 softmax denominators"):
                for sc in range(8):
                    if sc + 2 < 8:
                        nc.sync.dma_start(xt_sb[:, sc + 2], xt_d[sc + 2])
                    ssl = slice(sc * 512, (sc + 1) * 512)
                    psq = psb.tile([128, 512], f32, tag="ps512", name="psq")
                    for hc in range(8):
                        nc.tensor.matmul(psq, wq_sb[:, hc, :],
                                         xt_sb[:, sc, hc, :],
                                         start=(hc == 0), stop=(hc == 7))
                    nc.scalar.mul(qt_sb[:, ssl], psq, 0.125)

                    psk = psb.tile([128, 512], f32, tag="ps512", name="psk")
                    for hc in range(8):
                        nc.tensor.matmul(psk, wk_sb[:, hc, :],
                                         xt_sb[:, sc, hc, :],
                                         start=(hc == 0), stop=(hc == 7))
                    nc.vector.tensor_copy(kt_sb[:, ssl], psk)

                    # v transposed (512-free matmuls, ldweights hidden), then
                    # PE-transpose 128-blocks back to natural [token, dim]
                    psvT = psb.tile([128, 512], f32, tag="ps512", name="psvT")
                    for hc in range(8):
                        nc.tensor.matmul(psvT, wv_sb[:, hc, :],
                                         xt_sb[:, sc, hc, :],
                                         start=(hc == 0), stop=(hc == 7))
                    vt_sb = vt_p.tile([128, 512], bf16, tag="vt", name="vt")
                    nc.vector.tensor_copy(vt_sb, psvT)
                    for b in range(4):
                        kb = sc * 4 + b
                        psvt = psv.tile([128, 256], bf16, tag="psv",
                                        name="psvt")
                        nc.tensor.transpose(
                            psvt[:, 0:128], vt_sb[:, b * 128:b * 128 + 128],
                            id_sb)
                        vdst = v_sb[:, kb, :].rearrange("p (h c) -> p h c", h=2)
                        nc.vector.tensor_copy(
                            vdst[:, :, 0:64],
                            psvt[:, 0:128].rearrange("p (h c) -> p h c", h=2))

                    if sc == 0:
                        for j in range(2):
                            nc.vector.tensor_copy(
                                kts_sb[:, j, 32 * j:32 * j + 1], kt_sb[:, 0:1])
                        nc.gpsimd.partition_broadcast(v0rep_sb, v_sb[0:1, 0, :])

                    # band tiles unlocked by this projection chunk
                    hi_kb = min(4 * sc + 2, NKB - 2) if sc < 7 else NKB - 1
                    for h in range(2):
                        for kb in range(next_kb, hi_kb + 1):
                            emit_band(kb, h)
                    next_kb = hi_kb + 1

                    if sc % 2 == 1:
                        emit_strip_group(sc // 2)

                    if sc >= 1:
                        emit_pv_finalize(sc - 1)
                        if sc >= 2:
                            emit_outproj(sc - 2, outts.pop(sc - 2))
                # drain: last window + trailing out-projs
                emit_pv_finalize(NW - 1)
                emit_outproj(NW - 2, outts.pop(NW - 2))
                emit_outproj(NW - 1, outts.pop(NW - 1))

    nc.compile()
    return nc


def _host_row0(x, Wq, Wk, Wv, Wo):
    """Full-softmax attention output row for global query 0 (all 16 heads)."""
    f32 = np.float32
    xb = np.asarray(x)[0].astype(BF16)
    q0 = ((xb[0:1].astype(f32) @ np.asarray(Wq).astype(BF16).astype(f32).T)
          * f32(0.125)).astype(BF16).astype(f32)[0]
    K = (xb.astype(f32) @ np.asarray(Wk).astype(BF16).astype(f32).T
         ).astype(BF16).astype(f32)
    V = (xb.astype(f32) @ np.asarray(Wv).astype(BF16).astype(f32).T
         ).astype(BF16).astype(f32)
    out0 = np.empty(HIDDEN, f32)
    for h in range(16):
        sl = slice(64 * h, 64 * h + 64)
        s = K[:, sl] @ q0[sl]
        p = np.exp(s - s.max())
        out0[sl] = (p @ V[:, sl]) / p.sum()
    out0 = out0.astype(BF16).astype(f32)
    return out0 @ np.asarray(Wo).astype(BF16).astype(f32).T


def kernel(x, Wq, Wk, Wv, Wo):
    from concourse import bass_utils

    x = np.asarray(x)
    B = x.shape[0]
    xb = x[0].astype(BF16)
    xt = np.ascontiguousarray(
        xb.reshape(8, 512, 8, 128).transpose(0, 3, 2, 1))

    def wprep(W, rs):
        wt = np.asarray(W)[rs, :].T.astype(BF16)
        return np.ascontiguousarray(
            wt.reshape(8, 128, OC).transpose(1, 0, 2))

    in_maps = []
    for d in range(N_CORES):
        rs = slice(OC * d, OC * (d + 1))
        in_maps.append({
            "xt": xt,
            "wqt": wprep(Wq, rs),
            "wkt": wprep(Wk, rs),
            "wvt": wprep(Wv, rs),
            "wot": np.ascontiguousarray(np.asarray(Wo)[:, rs].T.astype(BF16)),
        })

    if "nc" not in _CACHE:
        _CACHE["nc"] = _build()
    nc = _CACHE["nc"]

    res = bass_utils.run_bass_kernel_spmd(
        nc, in_maps, core_ids=list(range(N_CORES)),
        trace=bool(os.environ.get("KERNEL_TRACE")))
    global LAST_RESULTS
    LAST_RESULTS = res

    out = np.zeros((S, HIDDEN), np.float64)
    for r in res.results:
        out += r["partial"].astype(np.float64)
    out[0, :] = _host_row0(x, Wq, Wk, Wv, Wo)
    return out.reshape(B, S, HIDDEN).astype(np.float32)


# revision 34
# speedup vs baseline: 1.2315x; 1.0374x over previous
"""Longformer attention TP-sharded Bass kernel for 8 NeuronCores (v3).

Sharding: tensor-parallel over heads. Core d owns heads 2d, 2d+1:
  - Wq/Wk/Wv rows [128d:128(d+1)]  (nn.Linear: q = x @ Wq.T)
  - Wo columns [128d:128(d+1)]
  Each core computes its heads' sparse (windowed+global) attention and a
  full-size out-proj partial; host sums the 8 partials.

v3: fully software-pipelined. Projections (phase A) are interleaved with
attention (phase B): after projection chunk sc (512 tokens), the band
score tiles it unlocks are emitted, strip scores on odd sc, and the
PV/normalize/out-proj for window sc-1. This keeps the PE continuously
busy (p-state stays high) and hides the ACT/DVE elementwise work that
would otherwise serialize phase B.

Per head layout:
  - band tile kb: ONE matmul scoresT [128k, 384q] over the query span
    [128(kb-1), 128(kb+2)); triangular masks post-exp on the outer
    128-col regions (multiplicative, scores are O(1) so no max needed).
  - PV transposed: psum_pv [65, 512q] accumulates lhsT=v_ones[128k, 65]
    x rhs=probsT chunks; row 64 (ones column) = softmax denominator.
  - k=0 global column handled as rank-1 "strips": s(0, q) packed two
    512-query windows per psum tile at partitions {0, 32} (PE quadrant
    anchors), one exp per group, outer-product PV matmuls. The strip is
    the unique start=True initializer of each PV bank (start resets the
    whole bank); the kb0 tile's key-0 row is masked off to compensate.
  - denominators: ACT copies psum row 64 to SBUF (the custom DVE
    reciprocal mis-addresses PSUM at partition base 64), DVE
    reciprocal_approx_fast + bf16 cast, PE outer-product broadcast to
    64 partitions, one DVE multiply -> outT feeds out-proj as lhsT.
  - q=0 global row is patched on the HOST (full-softmax row in numpy
    from bf16-cast inputs; 1 of 4096 rows).
"""

import os
import numpy as np
import ml_dtypes

S = 4096
HIDDEN = 1024
N_CORES = 8
OC = 128          # out-proj contraction dims (head dims) per core = 2 heads x 64
NKB = S // 128    # 32 key blocks
NW = S // 512     # 8 query windows
BF16 = ml_dtypes.bfloat16

_CACHE = {}
LAST_RESULTS = None


def _masks_np():
    """[4, 128, 128]: m_up (f>=p), m_lo (f<=p), m_nr0 (p>0),
    m_lo_nr0 (m_lo & p>0)."""
    p = np.arange(128)[:, None]   # key index within block (partition)
    f = np.arange(128)[None, :]   # query index within block (free)
    m_up = (f >= p)
    m_lo = (f <= p)
    m_nr0 = np.broadcast_to(p > 0, (128, 128))
    return np.stack([m_up, m_lo, m_nr0, m_lo & m_nr0]).astype(BF16)


def _band_clip(kb, w):
    """Overlap of tile kb's query span with window w, or None.
    Returns (psum_col_lo, psum_col_hi, tile_col_lo)."""
    s0 = 128 * (kb - 1)
    s1 = min(128 * (kb + 2), S)
    lo = max(512 * w, s0)
    hi = min(512 * w + 512, s1)
    if lo >= hi:
        return None
    return lo - 512 * w, hi - 512 * w, lo - s0


def _build():
    import concourse.bass as bass
    import concourse.mybir as mybir
    import concourse.tile as tile
    from concourse import bacc

    f32 = mybir.dt.float32
    bf16 = mybir.dt.bfloat16
    Exp = mybir.ActivationFunctionType.Exp

    nc = bacc.Bacc("TRN2", target_bir_lowering=False, debug=False,
                   num_devices=N_CORES)

    xt_d = nc.dram_tensor("xt", [8, 128, 8, 512], bf16, kind="ExternalInput").ap()
    wq_d = nc.dram_tensor("wqt", [128, 8, OC], bf16, kind="ExternalInput").ap()
    wk_d = nc.dram_tensor("wkt", [128, 8, OC], bf16, kind="ExternalInput").ap()
    wv_d = nc.dram_tensor("wvt", [128, 8, OC], bf16, kind="ExternalInput").ap()
    wo_d = nc.dram_tensor("wot", [OC, HIDDEN], bf16, kind="ExternalInput").ap()
    out_d = nc.dram_tensor("partial", [S, HIDDEN], bf16,
                           kind="ExternalOutput").ap()
    mask_d = nc.inline_tensor(_masks_np(), name="masks").ap()
    id_d = nc.inline_tensor(np.eye(128, dtype=BF16), name="ident").ap()

    with tile.TileContext(nc) as tc:
        import contextlib
        with contextlib.ExitStack() as ctx:
            big = ctx.enter_context(tc.tile_pool(name="big", bufs=1))
            probsp = ctx.enter_context(tc.tile_pool(name="probsp", bufs=20))
            tmp = ctx.enter_context(tc.tile_pool(name="tmp", bufs=2))
            outtp = ctx.enter_context(tc.tile_pool(name="outtp", bufs=2))
            stgp = ctx.enter_context(tc.tile_pool(name="stgp", bufs=2))
            psb = ctx.enter_context(tc.tile_pool(name="psb", bufs=3, space="PSUM"))
            psv = ctx.enter_context(tc.tile_pool(name="psv", bufs=1, space="PSUM"))
            pvp = ctx.enter_context(tc.tile_pool(name="pvp", bufs=2, space="PSUM"))
            opp = ctx.enter_context(tc.tile_pool(name="opp", bufs=2, space="PSUM"))

            # ---- resident tensors ----
            xt_sb = big.tile([128, 8, 8, 512], bf16)   # [p, sc, hc, s]
            qt_sb = big.tile([128, S], bf16)          # q.T * 0.125
            kt_sb = big.tile([128, S], bf16)
            v_sb = big.tile([128, NKB, 130], bf16)    # [vA|1|vB|1] per key block
            wq_sb = big.tile([128, 8, OC], bf16)
            wk_sb = big.tile([128, 8, OC], bf16)
            wv_sb = big.tile([128, 8, OC], bf16)
            wo_sb = big.tile([128, HIDDEN], bf16)
            mask_sb = big.tile([128, 4, 128], bf16)
            ones_sb = big.tile([1, 64], bf16)         # bcast lhsT
            id_sb = big.tile([128, 128], bf16)        # PE transpose identity
            vt_p = ctx.enter_context(tc.tile_pool(name="vt_p", bufs=2))
            kts_sb = big.tile([128, 2, 33], bf16)     # strip lhsT: kt0 at col 32j
            v0rep_sb = big.tile([33, 130], bf16)      # v row for key 0, replicated
            # packed strip probs: [group g][head h] -> [33, 512], rows 32j;
            # group g covers windows {2g, 2g+1}
            p0_sb = big.tile([33, 4, 2, 512], bf16)

            # ---- input DMAs: only the first two x chunks up front so chunk
            # 0 gets the full DMA bandwidth; the rest prefetch inside the
            # loop two iterations ahead ----
            nc.sync.dma_start(xt_sb[:, 0], xt_d[0])
            nc.sync.dma_start(xt_sb[:, 1], xt_d[1])
            nc.gpsimd.dma_start(wq_sb, wq_d)
            nc.sync.dma_start(wk_sb, wk_d)
            nc.scalar.dma_start(wv_sb, wv_d)
            nc.sync.dma_start(wo_sb, wo_d)
            nc.scalar.dma_start(mask_sb, mask_d.rearrange("m p f -> p m f"))
            nc.vector.memset(v_sb[:, :, 64], 1.0)
            nc.vector.memset(v_sb[:, :, 129], 1.0)
            nc.vector.memset(ones_sb, 1.0)
            nc.vector.memset(kts_sb, 0.0)
            nc.scalar.dma_start(id_sb, id_d)

            probs = {}

            def emit_band(kb, h):
                bp = 64 * h
                pt = probsp.tile([128, 384], bf16, tag="probs", name="probs")
                pss = psb.tile([128, 512], f32, tag="ps512", name="pss")
                if kb == 0:
                    nc.tensor.matmul(pss[:, 0:256], kt_sb[bp:bp + 64, 0:128],
                                     qt_sb[bp:bp + 64, 0:256],
                                     start=True, stop=True)
                    nc.scalar.activation(pt[:, 0:256], pss[:, 0:256], Exp)
                    # {0:128}=m_nr0 (drop key-0 row), {128:256}=m_lo_nr0
                    pr = pt[:, 0:256].rearrange("p (a b) -> p a b", b=128)
                    nc.gpsimd.tensor_mul(pr, pr, mask_sb[:, 2:4, :])
                else:
                    s0 = 128 * (kb - 1)
                    wdt = min(128 * (kb + 2), S) - s0
                    nc.tensor.matmul(pss[:, 0:wdt],
                                     kt_sb[bp:bp + 64, 128 * kb:128 * kb + 128],
                                     qt_sb[bp:bp + 64, s0:s0 + wdt],
                                     start=True, stop=True)
                    nc.scalar.activation(pt[:, 0:wdt], pss[:, 0:wdt], Exp)
                    if kb == NKB - 1:
                        nc.gpsimd.tensor_mul(pt[:, 0:128], pt[:, 0:128],
                                             mask_sb[:, 0, :])
                    else:
                        pr = pt.rearrange("p (a b) -> p a b", b=128)[:, 0::2, :]
                        nc.gpsimd.tensor_mul(pr, pr, mask_sb[:, 0:2, :])
                probs[(kb, h)] = pt

            def emit_strip_group(g):
                # windows 2g, 2g+1 at partition rows 0, 32; needs qt through
                # window 2g+1 -> call at sc = 2g+1
                for h in range(2):
                    bp = 64 * h
                    ps0 = psb.tile([33, 512], f32, tag="ps512", name="ps0")
                    for j in range(2):
                        w = 2 * g + j
                        nc.tensor.matmul(
                            ps0, kts_sb[bp:bp + 64, j, :],
                            qt_sb[bp:bp + 64, 512 * w:512 * w + 512],
                            start=(j == 0), stop=(j == 1))
                    nc.scalar.activation(p0_sb[:, g, h, :], ps0, Exp)

            def emit_outproj(w, outt):
                stg = stgp.tile([128, 4, HIDDEN], bf16, tag="stg", name="stg")
                for c in range(4):
                    for oc in range(2):
                        pso = opp.tile([128, 512], f32, tag="op", name="pso")
                        nc.tensor.matmul(pso, outt[:, 128 * c:128 * c + 128],
                                         wo_sb[:, 512 * oc:512 * oc + 512],
                                         start=True, stop=True)
                        dst = stg[:, c, 512 * oc:512 * oc + 512]
                        if (2 * c + oc) % 2 == 0:
                            nc.scalar.copy(dst, pso)
                        else:
                            nc.vector.tensor_copy(dst, pso)
                dst = out_d[512 * w:512 * w + 512, :].rearrange(
                    "(c p) o -> p c o", p=128)
                nc.sync.dma_start(dst, stg)

            outts = {}

            def emit_pv_finalize(w):
                pvs = []
                for h in range(2):
                    pv = pvp.tile([65, 512], f32, tag="pv", name="pv")
                    g, j = divmod(w, 2)
                    nc.tensor.matmul(
                        pv, v0rep_sb[32 * j:32 * j + 1, 65 * h:65 * h + 65],
                        p0_sb[32 * j:32 * j + 1, g, h, :],
                        start=True, stop=False, skip_group_check=True)
                    if w == 0:
                        nc.tensor.matmul(
                            pv[:, 0:256], v_sb[:, 0, 65 * h:65 * h + 65],
                            probs[(0, h)][:, 0:256],
                            start=False, stop=False, skip_group_check=True)
                    kbs = [kb for kb in range(max(1, 4 * w - 1),
                                              min(4 * w + 5, NKB))
                           if _band_clip(kb, w) is not None]
                    for i, kb in enumerate(kbs):
                        lo, hi, tl = _band_clip(kb, w)
                        nc.tensor.matmul(
                            pv[:, lo:hi], v_sb[:, kb, 65 * h:65 * h + 65],
                            probs[(kb, h)][:, tl:tl + hi - lo],
                            start=False, stop=(i == len(kbs) - 1),
                            skip_group_check=True)
                    pvs.append(pv)
                recips = []
                for h in range(2):
                    rcs = tmp.tile([1, 512], f32, tag="rcs", name="rcs")
                    nc.scalar.copy(rcs, pvs[h][64:65, :])
                    rcf = tmp.tile([1, 512], f32, tag="rcf", name="rcf")
                    nc.vector.reciprocal_approx_fast(rcf, rcs)
                    rcb = tmp.tile([1, 512], bf16, tag="rcb", name="rcb")
                    nc.vector.tensor_copy(rcb, rcf)
                    recips.append(rcb)
                outt = outtp.tile([128, 512], bf16, tag="outt", name="outt")
                for h in range(2):
                    psbc = opp.tile([128, 512], f32, tag="op", name="psbc")
                    nc.tensor.matmul(psbc[0:64, :], ones_sb, recips[h],
                                     start=True, stop=True)
                    bc = tmp.tile([64, 512], bf16, tag="bc", name="bc")
                    if h == 0:
                        nc.scalar.copy(bc, psbc[0:64, :])
                    else:
                        nc.vector.tensor_copy(bc, psbc[0:64, :])
                    nc.vector.tensor_mul(outt[64 * h:64 * h + 64, :],
                                         pvs[h][0:64, :], bc)
                outts[w] = outt

            # ---- fully pipelined main loop ----
            next_kb = 0
            with nc.allow_low_precision("bf16 softmax denominators"):
                for sc in range(8):
                    if sc + 2 < 8:
                        nc.sync.dma_start(xt_sb[:, sc + 2], xt_d[sc + 2])
                    ssl = slice(sc * 512, (sc + 1) * 512)
                    psq = psb.tile([128, 512], f32, tag="ps512", name="psq")
                    for hc in range(8):
                        nc.tensor.matmul(psq, wq_sb[:, hc, :],
                                         xt_sb[:, sc, hc, :],
                                         start=(hc == 0), stop=(hc == 7))
                    nc.scalar.mul(qt_sb[:, ssl], psq, 0.125)

                    psk = psb.tile([128, 512], f32, tag="ps512", name="psk")
                    for hc in range(8):
                        nc.tensor.matmul(psk, wk_sb[:, hc, :],
                                         xt_sb[:, sc, hc, :],
                                         start=(hc == 0), stop=(hc == 7))
                    nc.vector.tensor_copy(kt_sb[:, ssl], psk)

                    for b in range(4):
                        kb = sc * 4 + b
                        psvt = psv.tile([128, 128], f32, tag="psv", name="psvt")
                        for hc in range(8):
                            nc.tensor.matmul(psvt,
                                             xt_sb[:, sc, hc,
                                                   b * 128:b * 128 + 128],
                                             wv_sb[:, hc, :],
                                             start=(hc == 0), stop=(hc == 7))
                        vdst = v_sb[:, kb, :].rearrange("p (h c) -> p h c", h=2)
                        nc.vector.tensor_copy(
                            vdst[:, :, 0:64],
                            psvt.rearrange("p (h c) -> p h c", h=2))

                    if sc == 0:
                        for j in range(2):
                            nc.vector.tensor_copy(
                                kts_sb[:, j, 32 * j:32 * j + 1], kt_sb[:, 0:1])
                        nc.gpsimd.partition_broadcast(v0rep_sb, v_sb[0:1, 0, :])

                    # band tiles unlocked by this projection chunk
                    hi_kb = min(4 * sc + 2, NKB - 2) if sc < 7 else NKB - 1
                    for h in range(2):
                        for kb in range(next_kb, hi_kb + 1):
                            emit_band(kb, h)
                    next_kb = hi_kb + 1

                    if sc % 2 == 1:
                        emit_strip_group(sc // 2)

                    if sc >= 1:
                        emit_pv_finalize(sc - 1)
                        if sc >= 2:
                            emit_outproj(sc - 2, outts.pop(sc - 2))
                # drain: last window + trailing out-projs
                emit_pv_finalize(NW - 1)
                emit_outproj(NW - 2, outts.pop(NW - 2))
                emit_outproj(NW - 1, outts.pop(NW - 1))

    nc.compile()
    return nc


def _host_row0(x, Wq, Wk, Wv, Wo):
    """Full-softmax attention output row for global query 0 (all 16 heads)."""
    f32 = np.float32
    xb = np.asarray(x)[0].astype(BF16)
    q0 = ((xb[0:1].astype(f32) @ np.asarray(Wq).astype(BF16).astype(f32).T)
          * f32(0.125)).astype(BF16).astype(f32)[0]
    K = (xb.astype(f32) @ np.asarray(Wk).astype(BF16).astype(f32).T
         ).astype(BF16).astype(f32)
    V = (xb.astype(f32) @ np.asarray(Wv).astype(BF16).astype(f32).T
         ).astype(BF16).astype(f32)
    out0 = np.empty(HIDDEN, f32)
    for h in range(16):
        sl = slice(64 * h, 64 * h + 64)
        s = K[:, sl] @ q0[sl]
        p = np.exp(s - s.max())
        out0[sl] = (p @ V[:, sl]) / p.sum()
    out0 = out0.astype(BF16).astype(f32)
    return out0 @ np.asarray(Wo).astype(BF16).astype(f32).T


def kernel(x, Wq, Wk, Wv, Wo):
    from concourse import bass_utils

    x = np.asarray(x)
    B = x.shape[0]
    xb = x[0].astype(BF16)
    xt = np.ascontiguousarray(
        xb.reshape(8, 512, 8, 128).transpose(0, 3, 2, 1))

    def wprep(W, rs):
        wt = np.asarray(W)[rs, :].T.astype(BF16)
        return np.ascontiguousarray(
            wt.reshape(8, 128, OC).transpose(1, 0, 2))

    in_maps = []
    for d in range(N_CORES):
        rs = slice(OC * d, OC * (d + 1))
        in_maps.append({
            "xt": xt,
            "wqt": wprep(Wq, rs),
            "wkt": wprep(Wk, rs),
            "wvt": wprep(Wv, rs),
            "wot": np.ascontiguousarray(np.asarray(Wo)[:, rs].T.astype(BF16)),
        })

    if "nc" not in _CACHE:
        _CACHE["nc"] = _build()
    nc = _CACHE["nc"]

    res = bass_utils.run_bass_kernel_spmd(
        nc, in_maps, core_ids=list(range(N_CORES)),
        trace=bool(os.environ.get("KERNEL_TRACE")))
    global LAST_RESULTS
    LAST_RESULTS = res

    out = np.zeros((S, HIDDEN), np.float64)
    for r in res.results:
        out += r["partial"].astype(np.float64)
    out[0, :] = _host_row0(x, Wq, Wk, Wv, Wo)
    return out.reshape(B, S, HIDDEN).astype(np.float32)


# revision 38
# speedup vs baseline: 1.2913x; 1.0486x over previous
"""Longformer attention TP-sharded Bass kernel for 8 NeuronCores (v3).

Sharding: tensor-parallel over heads. Core d owns heads 2d, 2d+1:
  - Wq/Wk/Wv rows [128d:128(d+1)]  (nn.Linear: q = x @ Wq.T)
  - Wo columns [128d:128(d+1)]
  Each core computes its heads' sparse (windowed+global) attention and a
  full-size out-proj partial; host sums the 8 partials.

v3: fully software-pipelined. Projections (phase A) are interleaved with
attention (phase B): after projection chunk sc (512 tokens), the band
score tiles it unlocks are emitted, strip scores on odd sc, and the
PV/normalize/out-proj for window sc-1. This keeps the PE continuously
busy (p-state stays high) and hides the ACT/DVE elementwise work that
would otherwise serialize phase B.

Per head layout:
  - band tile kb: ONE matmul scoresT [128k, 384q] over the query span
    [128(kb-1), 128(kb+2)); triangular masks post-exp on the outer
    128-col regions (multiplicative, scores are O(1) so no max needed).
  - PV transposed: psum_pv [65, 512q] accumulates lhsT=v_ones[128k, 65]
    x rhs=probsT chunks; row 64 (ones column) = softmax denominator.
  - k=0 global column handled as rank-1 "strips": s(0, q) packed two
    512-query windows per psum tile at partitions {0, 32} (PE quadrant
    anchors), one exp per group, outer-product PV matmuls. The strip is
    the unique start=True initializer of each PV bank (start resets the
    whole bank); the kb0 tile's key-0 row is masked off to compensate.
  - denominators: ACT copies psum row 64 to SBUF (the custom DVE
    reciprocal mis-addresses PSUM at partition base 64), DVE
    reciprocal_approx_fast + bf16 cast, PE outer-product broadcast to
    64 partitions, one DVE multiply -> outT feeds out-proj as lhsT.
  - q=0 global row is patched on the HOST (full-softmax row in numpy
    from bf16-cast inputs; 1 of 4096 rows).
"""

import os
import numpy as np
import ml_dtypes

S = 4096
HIDDEN = 1024
N_CORES = 8
OC = 128          # out-proj contraction dims (head dims) per core = 2 heads x 64
NKB = S // 128    # 32 key blocks
NW = S // 512     # 8 query windows
BF16 = ml_dtypes.bfloat16

_CACHE = {}
LAST_RESULTS = None


def _masks_np():
    """[4, 128, 128]: m_up (f>=p), m_lo (f<=p), m_nr0 (p>0),
    m_lo_nr0 (m_lo & p>0)."""
    p = np.arange(128)[:, None]   # key index within block (partition)
    f = np.arange(128)[None, :]   # query index within block (free)
    m_up = (f >= p)
    m_lo = (f <= p)
    m_nr0 = np.broadcast_to(p > 0, (128, 128))
    return np.stack([m_up, m_lo, m_nr0, m_lo & m_nr0]).astype(BF16)


def _band_clip(kb, w):
    """Overlap of tile kb's query span with window w, or None.
    Returns (psum_col_lo, psum_col_hi, tile_col_lo)."""
    s0 = 128 * (kb - 1)
    s1 = min(128 * (kb + 2), S)
    lo = max(512 * w, s0)
    hi = min(512 * w + 512, s1)
    if lo >= hi:
        return None
    return lo - 512 * w, hi - 512 * w, lo - s0


def _build():
    import concourse.bass as bass
    import concourse.mybir as mybir
    import concourse.tile as tile
    from concourse import bacc

    f32 = mybir.dt.float32
    bf16 = mybir.dt.bfloat16
    Exp = mybir.ActivationFunctionType.Exp

    nc = bacc.Bacc("TRN2", target_bir_lowering=False, debug=False,
                   num_devices=N_CORES)

    xt_d = nc.dram_tensor("xt", [8, 128, 8, 512], bf16, kind="ExternalInput").ap()
    wq_d = nc.dram_tensor("wqt", [128, 8, OC], bf16, kind="ExternalInput").ap()
    wk_d = nc.dram_tensor("wkt", [128, 8, OC], bf16, kind="ExternalInput").ap()
    wv_d = nc.dram_tensor("wvt", [128, 8, OC], bf16, kind="ExternalInput").ap()
    wo_d = nc.dram_tensor("wot", [OC, HIDDEN], bf16, kind="ExternalInput").ap()
    out_d = nc.dram_tensor("partial", [S, HIDDEN], bf16,
                           kind="ExternalOutput").ap()
    mask_d = nc.inline_tensor(_masks_np(), name="masks").ap()
    id_d = nc.inline_tensor(np.eye(128, dtype=BF16), name="ident").ap()

    with tile.TileContext(nc) as tc:
        import contextlib
        with contextlib.ExitStack() as ctx:
            big = ctx.enter_context(tc.tile_pool(name="big", bufs=1))
            probsp = ctx.enter_context(tc.tile_pool(name="probsp", bufs=20))
            tmp = ctx.enter_context(tc.tile_pool(name="tmp", bufs=2))
            outtp = ctx.enter_context(tc.tile_pool(name="outtp", bufs=2))
            stgp = ctx.enter_context(tc.tile_pool(name="stgp", bufs=2))
            psb = ctx.enter_context(tc.tile_pool(name="psb", bufs=3, space="PSUM"))
            psv = ctx.enter_context(tc.tile_pool(name="psv", bufs=1, space="PSUM"))
            pvp = ctx.enter_context(tc.tile_pool(name="pvp", bufs=2, space="PSUM"))
            opp = ctx.enter_context(tc.tile_pool(name="opp", bufs=2, space="PSUM"))

            # ---- resident tensors ----
            xt_sb = big.tile([128, 8, 8, 512], bf16)   # [p, sc, hc, s]
            qt_sb = big.tile([128, S], bf16)          # q.T * 0.125
            kt_sb = big.tile([128, S], bf16)
            v_sb = big.tile([128, NKB, 130], bf16)    # [vA|1|vB|1] per key block
            wq_sb = big.tile([128, 8, OC], bf16)
            wk_sb = big.tile([128, 8, OC], bf16)
            wv_sb = big.tile([128, 8, OC], bf16)
            wo_sb = big.tile([128, HIDDEN], bf16)
            mask_sb = big.tile([128, 4, 128], bf16)
            ones_sb = big.tile([1, 64], bf16)         # bcast lhsT
            id_sb = big.tile([128, 128], bf16)        # PE transpose identity
            vt_p = ctx.enter_context(tc.tile_pool(name="vt_p", bufs=2))
            kts_sb = big.tile([128, 2, 33], bf16)     # strip lhsT: kt0 at col 32j
            v0rep_sb = big.tile([33, 130], bf16)      # v row for key 0, replicated
            # packed strip probs: [group g][head h] -> [33, 512], rows 32j;
            # group g covers windows {2g, 2g+1}
            p0_sb = big.tile([33, 4, 2, 512], bf16)

            # ---- input DMAs: only the first two x chunks up front so chunk
            # 0 gets the full DMA bandwidth; the rest prefetch inside the
            # loop two iterations ahead ----
            nc.sync.dma_start(xt_sb[:, 0], xt_d[0])
            nc.sync.dma_start(xt_sb[:, 1], xt_d[1])
            nc.gpsimd.dma_start(wq_sb, wq_d)
            nc.sync.dma_start(wk_sb, wk_d)
            nc.scalar.dma_start(wv_sb, wv_d)
            nc.sync.dma_start(wo_sb, wo_d)
            nc.scalar.dma_start(mask_sb, mask_d.rearrange("m p f -> p m f"))
            nc.vector.memset(v_sb[:, :, 64], 1.0)
            nc.vector.memset(v_sb[:, :, 129], 1.0)
            nc.vector.memset(ones_sb, 1.0)
            nc.vector.memset(kts_sb, 0.0)
            nc.scalar.dma_start(id_sb, id_d)

            probs = {}

            def emit_band(kb, h):
                bp = 64 * h
                pt = probsp.tile([128, 384], bf16, tag="probs", name="probs")
                pss = psb.tile([128, 512], f32, tag="ps512", name="pss")
                if kb == 0:
                    nc.tensor.matmul(pss[:, 0:256], kt_sb[bp:bp + 64, 0:128],
                                     qt_sb[bp:bp + 64, 0:256],
                                     start=True, stop=True)
                    nc.scalar.activation(pt[:, 0:256], pss[:, 0:256], Exp)
                    # {0:128}=m_nr0 (drop key-0 row), {128:256}=m_lo_nr0
                    pr = pt[:, 0:256].rearrange("p (a b) -> p a b", b=128)
                    nc.gpsimd.tensor_mul(pr, pr, mask_sb[:, 2:4, :])
                else:
                    s0 = 128 * (kb - 1)
                    wdt = min(128 * (kb + 2), S) - s0
                    nc.tensor.matmul(pss[:, 0:wdt],
                                     kt_sb[bp:bp + 64, 128 * kb:128 * kb + 128],
                                     qt_sb[bp:bp + 64, s0:s0 + wdt],
                                     start=True, stop=True)
                    nc.scalar.activation(pt[:, 0:wdt], pss[:, 0:wdt], Exp)
                    if kb == NKB - 1:
                        nc.gpsimd.tensor_mul(pt[:, 0:128], pt[:, 0:128],
                                             mask_sb[:, 0, :])
                    else:
                        pr = pt.rearrange("p (a b) -> p a b", b=128)[:, 0::2, :]
                        nc.gpsimd.tensor_mul(pr, pr, mask_sb[:, 0:2, :])
                probs[(kb, h)] = pt

            def emit_strip_group(g):
                # windows 2g, 2g+1 at partition rows 0, 32; needs qt through
                # window 2g+1 -> call at sc = 2g+1
                for h in range(2):
                    bp = 64 * h
                    ps0 = psb.tile([33, 512], f32, tag="ps512", name="ps0")
                    for j in range(2):
                        w = 2 * g + j
                        nc.tensor.matmul(
                            ps0, kts_sb[bp:bp + 64, j, :],
                            qt_sb[bp:bp + 64, 512 * w:512 * w + 512],
                            start=(j == 0), stop=(j == 1))
                    nc.scalar.activation(p0_sb[:, g, h, :], ps0, Exp)

            def emit_outproj(w, outt):
                stg = stgp.tile([128, 4, HIDDEN], bf16, tag="stg", name="stg")
                for c in range(4):
                    for oc in range(2):
                        pso = opp.tile([128, 512], f32, tag="op", name="pso")
                        nc.tensor.matmul(pso, outt[:, 128 * c:128 * c + 128],
                                         wo_sb[:, 512 * oc:512 * oc + 512],
                                         start=True, stop=True)
                        dst = stg[:, c, 512 * oc:512 * oc + 512]
                        if (2 * c + oc) % 2 == 0:
                            nc.scalar.copy(dst, pso)
                        else:
                            nc.vector.tensor_copy(dst, pso)
                    if c % 2 == 1:
                        half = c // 2
                        rows = slice(512 * w + 256 * half,
                                     512 * w + 256 * half + 256)
                        dst = out_d[rows, :].rearrange("(c p) o -> p c o",
                                                       p=128)
                        nc.sync.dma_start(dst, stg[:, 2 * half:2 * half + 2, :])

            outts = {}

            def emit_pv_finalize(w):
                pvs = []
                for h in range(2):
                    pv = pvp.tile([65, 512], f32, tag="pv", name="pv")
                    g, j = divmod(w, 2)
                    nc.tensor.matmul(
                        pv, v0rep_sb[32 * j:32 * j + 1, 65 * h:65 * h + 65],
                        p0_sb[32 * j:32 * j + 1, g, h, :],
                        start=True, stop=False, skip_group_check=True)
                    if w == 0:
                        nc.tensor.matmul(
                            pv[:, 0:256], v_sb[:, 0, 65 * h:65 * h + 65],
                            probs[(0, h)][:, 0:256],
                            start=False, stop=False, skip_group_check=True)
                    kbs = [kb for kb in range(max(1, 4 * w - 1),
                                              min(4 * w + 5, NKB))
                           if _band_clip(kb, w) is not None]
                    for i, kb in enumerate(kbs):
                        lo, hi, tl = _band_clip(kb, w)
                        nc.tensor.matmul(
                            pv[:, lo:hi], v_sb[:, kb, 65 * h:65 * h + 65],
                            probs[(kb, h)][:, tl:tl + hi - lo],
                            start=False, stop=(i == len(kbs) - 1),
                            skip_group_check=True)
                    pvs.append(pv)
                recips = []
                for h in range(2):
                    rcs = tmp.tile([1, 512], f32, tag="rcs", name="rcs")
                    if h == 0:
                        nc.scalar.copy(rcs, pvs[h][64:65, :])
                    else:
                        nc.vector.tensor_copy(rcs, pvs[h][64:65, :])
                    rcf = tmp.tile([1, 512], f32, tag="rcf", name="rcf")
                    nc.vector.reciprocal_approx_fast(rcf, rcs)
                    rcb = tmp.tile([1, 512], bf16, tag="rcb", name="rcb")
                    nc.vector.tensor_copy(rcb, rcf)
                    recips.append(rcb)
                outt = outtp.tile([128, 512], bf16, tag="outt", name="outt")
                for h in range(2):
                    psbc = opp.tile([128, 512], f32, tag="op", name="psbc")
                    nc.tensor.matmul(psbc[0:64, :], ones_sb, recips[h],
                                     start=True, stop=True)
                    bc = tmp.tile([64, 512], bf16, tag="bc", name="bc")
                    if h == 0:
                        nc.scalar.copy(bc, psbc[0:64, :])
                    else:
                        nc.vector.tensor_copy(bc, psbc[0:64, :])
                    nc.vector.tensor_mul(outt[64 * h:64 * h + 64, :],
                                         pvs[h][0:64, :], bc)
                outts[w] = outt

            # ---- fully pipelined main loop ----
            next_kb = 0
            with nc.allow_low_precision("bf16 softmax denominators"):
                for sc in range(8):
                    if sc + 2 < 8:
                        nc.sync.dma_start(xt_sb[:, sc + 2], xt_d[sc + 2])
                    ssl = slice(sc * 512, (sc + 1) * 512)
                    psq = psb.tile([128, 512], f32, tag="ps512", name="psq")
                    for hc in range(8):
                        nc.tensor.matmul(psq, wq_sb[:, hc, :],
                                         xt_sb[:, sc, hc, :],
                                         start=(hc == 0), stop=(hc == 7))
                    nc.scalar.mul(qt_sb[:, ssl], psq, 0.125)

                    psk = psb.tile([128, 512], f32, tag="ps512", name="psk")
                    for hc in range(8):
                        nc.tensor.matmul(psk, wk_sb[:, hc, :],
                                         xt_sb[:, sc, hc, :],
                                         start=(hc == 0), stop=(hc == 7))
                    nc.vector.tensor_copy(kt_sb[:, ssl], psk)

                    for b in range(4):
                        kb = sc * 4 + b
                        psvt = psv.tile([128, 128], f32, tag="psv", name="psvt")
                        for hc in range(8):
                            nc.tensor.matmul(psvt,
                                             xt_sb[:, sc, hc,
                                                   b * 128:b * 128 + 128],
                                             wv_sb[:, hc, :],
                                             start=(hc == 0), stop=(hc == 7))
                        vdst = v_sb[:, kb, :].rearrange("p (h c) -> p h c", h=2)
                        nc.vector.tensor_copy(
                            vdst[:, :, 0:64],
                            psvt.rearrange("p (h c) -> p h c", h=2))

                    if sc == 0:
                        for j in range(2):
                            nc.vector.tensor_copy(
                                kts_sb[:, j, 32 * j:32 * j + 1], kt_sb[:, 0:1])
                        nc.gpsimd.partition_broadcast(v0rep_sb, v_sb[0:1, 0, :])

                    # band tiles unlocked by this projection chunk
                    hi_kb = min(4 * sc + 2, NKB - 2) if sc < 7 else NKB - 1
                    for h in range(2):
                        for kb in range(next_kb, hi_kb + 1):
                            emit_band(kb, h)
                    next_kb = hi_kb + 1

                    if sc % 2 == 1:
                        emit_strip_group(sc // 2)

                    if sc >= 1:
                        emit_pv_finalize(sc - 1)
                        if sc >= 2:
                            emit_outproj(sc - 2, outts.pop(sc - 2))
                # drain: out-proj of window 6 first (its deps are already
                # met), overlapping window 7's PV chain on the other engines
                emit_outproj(NW - 2, outts.pop(NW - 2))
                emit_pv_finalize(NW - 1)
                emit_outproj(NW - 1, outts.pop(NW - 1))

    nc.compile()
    return nc


def _host_row0(x, Wq, Wk, Wv, Wo):
    """Full-softmax attention output row for global query 0 (all 16 heads)."""
    f32 = np.float32
    xb = np.asarray(x)[0].astype(BF16)
    q0 = ((xb[0:1].astype(f32) @ np.asarray(Wq).astype(BF16).astype(f32).T)
          * f32(0.125)).astype(BF16).astype(f32)[0]
    K = (xb.astype(f32) @ np.asarray(Wk).astype(BF16).astype(f32).T
         ).astype(BF16).astype(f32)
    V = (xb.astype(f32) @ np.asarray(Wv).astype(BF16).astype(f32).T
         ).astype(BF16).astype(f32)
    out0 = np.empty(HIDDEN, f32)
    for h in range(16):
        sl = slice(64 * h, 64 * h + 64)
        s = K[:, sl] @ q0[sl]
        p = np.exp(s - s.max())
        out0[sl] = (p @ V[:, sl]) / p.sum()
    out0 = out0.astype(BF16).astype(f32)
    return out0 @ np.asarray(Wo).astype(BF16).astype(f32).T


def kernel(x, Wq, Wk, Wv, Wo):
    from concourse import bass_utils

    x = np.asarray(x)
    B = x.shape[0]
    xb = x[0].astype(BF16)
    xt = np.ascontiguousarray(
        xb.reshape(8, 512, 8, 128).transpose(0, 3, 2, 1))

    def wprep(W, rs):
        wt = np.asarray(W)[rs, :].T.astype(BF16)
        return np.ascontiguousarray(
            wt.reshape(8, 128, OC).transpose(1, 0, 2))

    in_maps = []
    for d in range(N_CORES):
        rs = slice(OC * d, OC * (d + 1))
        in_maps.append({
            "xt": xt,
            "wqt": wprep(Wq, rs),
            "wkt": wprep(Wk, rs),
            "wvt": wprep(Wv, rs),
            "wot": np.ascontiguousarray(np.asarray(Wo)[:, rs].T.astype(BF16)),
        })

    if "nc" not in _CACHE:
        _CACHE["nc"] = _build()
    nc = _CACHE["nc"]

    res = bass_utils.run_bass_kernel_spmd(
        nc, in_maps, core_ids=list(range(N_CORES)),
        trace=bool(os.environ.get("KERNEL_TRACE")))
    global LAST_RESULTS
    LAST_RESULTS = res

    out = np.zeros((S, HIDDEN), np.float64)
    for r in res.results:
        out += r["partial"].astype(np.float64)
    out[0, :] = _host_row0(x, Wq, Wk, Wv, Wo)
    return out.reshape(B, S, HIDDEN).astype(np.float32)


# revision 41
# speedup vs baseline: 1.3741x; 1.0641x over previous
"""Longformer attention TP-sharded Bass kernel for 8 NeuronCores (v3).

Sharding: tensor-parallel over heads. Core d owns heads 2d, 2d+1:
  - Wq/Wk/Wv rows [128d:128(d+1)]  (nn.Linear: q = x @ Wq.T)
  - Wo columns [128d:128(d+1)]
  Each core computes its heads' sparse (windowed+global) attention and a
  full-size out-proj partial; host sums the 8 partials.

v3: fully software-pipelined. Projections (phase A) are interleaved with
attention (phase B): after projection chunk sc (512 tokens), the band
score tiles it unlocks are emitted, strip scores on odd sc, and the
PV/normalize/out-proj for window sc-1. This keeps the PE continuously
busy (p-state stays high) and hides the ACT/DVE elementwise work that
would otherwise serialize phase B.

Per head layout:
  - band tile kb: ONE matmul scoresT [128k, 384q] over the query span
    [128(kb-1), 128(kb+2)); triangular masks post-exp on the outer
    128-col regions (multiplicative, scores are O(1) so no max needed).
  - PV transposed: psum_pv [65, 512q] accumulates lhsT=v_ones[128k, 65]
    x rhs=probsT chunks; row 64 (ones column) = softmax denominator.
  - k=0 global column handled as rank-1 "strips": s(0, q) packed two
    512-query windows per psum tile at partitions {0, 32} (PE quadrant
    anchors), one exp per group, outer-product PV matmuls. The strip is
    the unique start=True initializer of each PV bank (start resets the
    whole bank); the kb0 tile's key-0 row is masked off to compensate.
  - denominators: ACT copies psum row 64 to SBUF (the custom DVE
    reciprocal mis-addresses PSUM at partition base 64), DVE
    reciprocal_approx_fast + bf16 cast, PE outer-product broadcast to
    64 partitions, one DVE multiply -> outT feeds out-proj as lhsT.
  - q=0 global row is patched on the HOST (full-softmax row in numpy
    from bf16-cast inputs; 1 of 4096 rows).
"""

import os
import numpy as np
import ml_dtypes

S = 4096
HIDDEN = 1024
N_CORES = 8
OC = 128          # out-proj contraction dims (head dims) per core = 2 heads x 64
NKB = S // 128    # 32 key blocks
NW = S // 512     # 8 query windows
BF16 = ml_dtypes.bfloat16

_CACHE = {}
LAST_RESULTS = None


def _masks_np():
    """[4, 128, 128]: m_up (f>=p), m_lo (f<=p), m_nr0 (p>0),
    m_lo_nr0 (m_lo & p>0)."""
    p = np.arange(128)[:, None]   # key index within block (partition)
    f = np.arange(128)[None, :]   # query index within block (free)
    m_up = (f >= p)
    m_lo = (f <= p)
    m_nr0 = np.broadcast_to(p > 0, (128, 128))
    return np.stack([m_up, m_lo, m_nr0, m_lo & m_nr0]).astype(BF16)


def _band_clip(kb, w):
    """Overlap of tile kb's query span with window w, or None.
    Returns (psum_col_lo, psum_col_hi, tile_col_lo)."""
    s0 = 128 * (kb - 1)
    s1 = min(128 * (kb + 2), S)
    lo = max(512 * w, s0)
    hi = min(512 * w + 512, s1)
    if lo >= hi:
        return None
    return lo - 512 * w, hi - 512 * w, lo - s0


def _build():
    import concourse.bass as bass
    import concourse.mybir as mybir
    import concourse.tile as tile
    from concourse import bacc

    f32 = mybir.dt.float32
    bf16 = mybir.dt.bfloat16
    Exp = mybir.ActivationFunctionType.Exp

    nc = bacc.Bacc("TRN2", target_bir_lowering=False, debug=False,
                   num_devices=N_CORES)

    xt_d = nc.dram_tensor("xt", [8, 128, 8, 512], bf16, kind="ExternalInput").ap()
    wq_d = nc.dram_tensor("wqt", [128, 8, OC], bf16, kind="ExternalInput").ap()
    wk_d = nc.dram_tensor("wkt", [128, 8, OC], bf16, kind="ExternalInput").ap()
    wv_d = nc.dram_tensor("wvt", [128, 8, OC], bf16, kind="ExternalInput").ap()
    wo_d = nc.dram_tensor("wot", [OC, HIDDEN], bf16, kind="ExternalInput").ap()
    out_d = nc.dram_tensor("partial", [S, HIDDEN], bf16,
                           kind="ExternalOutput").ap()
    mask_d = nc.inline_tensor(_masks_np(), name="masks").ap()
    id_d = nc.inline_tensor(np.eye(128, dtype=BF16), name="ident").ap()

    with tile.TileContext(nc) as tc:
        import contextlib
        with contextlib.ExitStack() as ctx:
            big = ctx.enter_context(tc.tile_pool(name="big", bufs=1))
            probsp = ctx.enter_context(tc.tile_pool(name="probsp", bufs=24))
            tmp = ctx.enter_context(tc.tile_pool(name="tmp", bufs=3))
            outtp = ctx.enter_context(tc.tile_pool(name="outtp", bufs=3))
            stgp = ctx.enter_context(tc.tile_pool(name="stgp", bufs=3))
            psb = ctx.enter_context(tc.tile_pool(name="psb", bufs=3, space="PSUM"))
            psv = ctx.enter_context(tc.tile_pool(name="psv", bufs=1, space="PSUM"))
            pvp = ctx.enter_context(tc.tile_pool(name="pvp", bufs=2, space="PSUM"))
            opp = ctx.enter_context(tc.tile_pool(name="opp", bufs=2, space="PSUM"))

            # ---- resident tensors ----
            xt_sb = big.tile([128, 8, 8, 512], bf16)   # [p, sc, hc, s]
            qt_sb = big.tile([128, S], bf16)          # q.T * 0.125
            kt_sb = big.tile([128, S], bf16)
            v_sb = big.tile([128, NKB, 130], bf16)    # [vA|1|vB|1] per key block
            wq_sb = big.tile([128, 8, OC], bf16)
            wk_sb = big.tile([128, 8, OC], bf16)
            wv_sb = big.tile([128, 8, OC], bf16)
            wo_sb = big.tile([128, HIDDEN], bf16)
            mask_sb = big.tile([128, 4, 128], bf16)
            ones_sb = big.tile([1, 64], bf16)         # bcast lhsT
            id_sb = big.tile([128, 128], bf16)        # PE transpose identity
            vt_p = ctx.enter_context(tc.tile_pool(name="vt_p", bufs=2))
            kts_sb = big.tile([128, 2, 33], bf16)     # strip lhsT: kt0 at col 32j
            v0rep_sb = big.tile([33, 130], bf16)      # v row for key 0, replicated
            # packed strip probs: [group g][head h] -> [33, 512], rows 32j;
            # group g covers windows {2g, 2g+1}
            p0_sb = big.tile([33, 4, 2, 512], bf16)

            # ---- input DMAs: only the first two x chunks up front so chunk
            # 0 gets the full DMA bandwidth; the rest prefetch inside the
            # loop two iterations ahead ----
            nc.sync.dma_start(xt_sb[:, 0], xt_d[0])
            nc.sync.dma_start(xt_sb[:, 1], xt_d[1])
            nc.gpsimd.dma_start(wq_sb, wq_d)
            nc.sync.dma_start(wk_sb, wk_d)
            nc.scalar.dma_start(wv_sb, wv_d)
            nc.sync.dma_start(wo_sb, wo_d)
            nc.scalar.dma_start(mask_sb, mask_d.rearrange("m p f -> p m f"))
            nc.vector.memset(v_sb[:, :, 64], 1.0)
            nc.vector.memset(v_sb[:, :, 129], 1.0)
            nc.vector.memset(ones_sb, 1.0)
            nc.vector.memset(kts_sb, 0.0)
            nc.scalar.dma_start(id_sb, id_d)

            probs = {}

            def emit_band(kb, h):
                bp = 64 * h
                pt = probsp.tile([128, 384], bf16, tag="probs", name="probs")
                pss = psb.tile([128, 512], f32, tag="ps512", name="pss")
                if kb == 0:
                    nc.tensor.matmul(pss[:, 0:256], kt_sb[bp:bp + 64, 0:128],
                                     qt_sb[bp:bp + 64, 0:256],
                                     start=True, stop=True)
                    nc.scalar.activation(pt[:, 0:256], pss[:, 0:256], Exp)
                    # {0:128}=m_nr0 (drop key-0 row), {128:256}=m_lo_nr0
                    pr = pt[:, 0:256].rearrange("p (a b) -> p a b", b=128)
                    meng = nc.vector if h == 0 else nc.gpsimd
                    meng.tensor_mul(pr, pr, mask_sb[:, 2:4, :])
                else:
                    s0 = 128 * (kb - 1)
                    wdt = min(128 * (kb + 2), S) - s0
                    nc.tensor.matmul(pss[:, 0:wdt],
                                     kt_sb[bp:bp + 64, 128 * kb:128 * kb + 128],
                                     qt_sb[bp:bp + 64, s0:s0 + wdt],
                                     start=True, stop=True)
                    nc.scalar.activation(pt[:, 0:wdt], pss[:, 0:wdt], Exp)
                    meng = nc.vector if h == 0 else nc.gpsimd
                    if kb == NKB - 1:
                        meng.tensor_mul(pt[:, 0:128], pt[:, 0:128],
                                        mask_sb[:, 0, :])
                    else:
                        pr = pt.rearrange("p (a b) -> p a b", b=128)[:, 0::2, :]
                        meng.tensor_mul(pr, pr, mask_sb[:, 0:2, :])
                probs[(kb, h)] = pt

            def emit_strip_group(g):
                # windows 2g, 2g+1 at partition rows 0, 32; needs qt through
                # window 2g+1 -> call at sc = 2g+1
                for h in range(2):
                    bp = 64 * h
                    ps0 = psb.tile([33, 512], f32, tag="ps512", name="ps0")
                    for j in range(2):
                        w = 2 * g + j
                        nc.tensor.matmul(
                            ps0, kts_sb[bp:bp + 64, j, :],
                            qt_sb[bp:bp + 64, 512 * w:512 * w + 512],
                            start=(j == 0), stop=(j == 1))
                    nc.scalar.activation(p0_sb[:, g, h, :], ps0, Exp)

            def emit_outproj(w, outt):
                stg = stgp.tile([128, 4, HIDDEN], bf16, tag="stg", name="stg")
                for c in range(4):
                    for oc in range(2):
                        pso = opp.tile([128, 512], f32, tag="op", name="pso")
                        nc.tensor.matmul(pso, outt[:, 128 * c:128 * c + 128],
                                         wo_sb[:, 512 * oc:512 * oc + 512],
                                         start=True, stop=True)
                        dst = stg[:, c, 512 * oc:512 * oc + 512]
                        if (2 * c + oc) % 2 == 0:
                            nc.scalar.copy(dst, pso)
                        else:
                            nc.vector.tensor_copy(dst, pso)
                    if c % 2 == 1:
                        half = c // 2
                        rows = slice(512 * w + 256 * half,
                                     512 * w + 256 * half + 256)
                        dst = out_d[rows, :].rearrange("(c p) o -> p c o",
                                                       p=128)
                        nc.sync.dma_start(dst, stg[:, 2 * half:2 * half + 2, :])

            outts = {}

            def emit_pv_finalize(w):
                pvs = []
                for h in range(2):
                    pv = pvp.tile([65, 512], f32, tag="pv", name="pv")
                    g, j = divmod(w, 2)
                    nc.tensor.matmul(
                        pv, v0rep_sb[32 * j:32 * j + 1, 65 * h:65 * h + 65],
                        p0_sb[32 * j:32 * j + 1, g, h, :],
                        start=True, stop=False, skip_group_check=True)
                    if w == 0:
                        nc.tensor.matmul(
                            pv[:, 0:256], v_sb[:, 0, 65 * h:65 * h + 65],
                            probs[(0, h)][:, 0:256],
                            start=False, stop=False, skip_group_check=True)
                    kbs = [kb for kb in range(max(1, 4 * w - 1),
                                              min(4 * w + 5, NKB))
                           if _band_clip(kb, w) is not None]
                    for i, kb in enumerate(kbs):
                        lo, hi, tl = _band_clip(kb, w)
                        nc.tensor.matmul(
                            pv[:, lo:hi], v_sb[:, kb, 65 * h:65 * h + 65],
                            probs[(kb, h)][:, tl:tl + hi - lo],
                            start=False, stop=(i == len(kbs) - 1),
                            skip_group_check=True)
                    pvs.append(pv)
                recips = []
                for h in range(2):
                    rcs = tmp.tile([1, 512], f32, tag="rcs", name="rcs")
                    if h == 0:
                        nc.scalar.copy(rcs, pvs[h][64:65, :])
                    else:
                        nc.vector.tensor_copy(rcs, pvs[h][64:65, :])
                    rcf = tmp.tile([1, 512], f32, tag="rcf", name="rcf")
                    nc.vector.reciprocal_approx_fast(rcf, rcs)
                    rcb = tmp.tile([1, 512], bf16, tag="rcb", name="rcb")
                    nc.vector.tensor_copy(rcb, rcf)
                    recips.append(rcb)
                outt = outtp.tile([128, 512], bf16, tag="outt", name="outt")
                for h in range(2):
                    psbc = opp.tile([128, 512], f32, tag="op", name="psbc")
                    nc.tensor.matmul(psbc[0:64, :], ones_sb, recips[h],
                                     start=True, stop=True)
                    bc = tmp.tile([64, 512], bf16, tag="bc", name="bc")
                    if h == 0:
                        nc.scalar.copy(bc, psbc[0:64, :])
                    else:
                        nc.vector.tensor_copy(bc, psbc[0:64, :])
                    nc.vector.tensor_mul(outt[64 * h:64 * h + 64, :],
                                         pvs[h][0:64, :], bc)
                outts[w] = outt

            # ---- fully pipelined main loop ----
            next_kb = 0
            with nc.allow_low_precision("bf16 softmax denominators"):
                for sc in range(8):
                    if sc + 2 < 8:
                        nc.sync.dma_start(xt_sb[:, sc + 2], xt_d[sc + 2])
                    ssl = slice(sc * 512, (sc + 1) * 512)
                    psq = psb.tile([128, 512], f32, tag="ps512", name="psq")
                    for hc in range(8):
                        nc.tensor.matmul(psq, wq_sb[:, hc, :],
                                         xt_sb[:, sc, hc, :],
                                         start=(hc == 0), stop=(hc == 7))
                    nc.scalar.mul(qt_sb[:, ssl], psq, 0.125)

                    psk = psb.tile([128, 512], f32, tag="ps512", name="psk")
                    for hc in range(8):
                        nc.tensor.matmul(psk, wk_sb[:, hc, :],
                                         xt_sb[:, sc, hc, :],
                                         start=(hc == 0), stop=(hc == 7))
                    nc.vector.tensor_copy(kt_sb[:, ssl], psk)

                    for b in range(4):
                        kb = sc * 4 + b
                        psvt = psv.tile([128, 128], f32, tag="psv", name="psvt")
                        for hc in range(8):
                            nc.tensor.matmul(psvt,
                                             xt_sb[:, sc, hc,
                                                   b * 128:b * 128 + 128],
                                             wv_sb[:, hc, :],
                                             start=(hc == 0), stop=(hc == 7))
                        vdst = v_sb[:, kb, :].rearrange("p (h c) -> p h c", h=2)
                        nc.vector.tensor_copy(
                            vdst[:, :, 0:64],
                            psvt.rearrange("p (h c) -> p h c", h=2))

                    if sc == 0:
                        for j in range(2):
                            nc.vector.tensor_copy(
                                kts_sb[:, j, 32 * j:32 * j + 1], kt_sb[:, 0:1])
                        nc.gpsimd.partition_broadcast(v0rep_sb, v_sb[0:1, 0, :])

                    # band tiles unlocked by this projection chunk
                    hi_kb = min(4 * sc + 2, NKB - 2) if sc < 7 else NKB - 1
                    for h in range(2):
                        for kb in range(next_kb, hi_kb + 1):
                            emit_band(kb, h)
                    next_kb = hi_kb + 1

                    if sc % 2 == 1:
                        emit_strip_group(sc // 2)

                    if sc >= 1:
                        emit_pv_finalize(sc - 1)
                        if sc >= 2:
                            emit_outproj(sc - 2, outts.pop(sc - 2))
                # drain: out-proj of window 6 first (its deps are already
                # met), overlapping window 7's PV chain on the other engines
                emit_outproj(NW - 2, outts.pop(NW - 2))
                emit_pv_finalize(NW - 1)
                emit_outproj(NW - 1, outts.pop(NW - 1))

    nc.compile()
    return nc


def _host_row0(x, Wq, Wk, Wv, Wo):
    """Full-softmax attention output row for global query 0 (all 16 heads)."""
    f32 = np.float32
    xb = np.asarray(x)[0].astype(BF16)
    q0 = ((xb[0:1].astype(f32) @ np.asarray(Wq).astype(BF16).astype(f32).T)
          * f32(0.125)).astype(BF16).astype(f32)[0]
    K = (xb.astype(f32) @ np.asarray(Wk).astype(BF16).astype(f32).T
         ).astype(BF16).astype(f32)
    V = (xb.astype(f32) @ np.asarray(Wv).astype(BF16).astype(f32).T
         ).astype(BF16).astype(f32)
    out0 = np.empty(HIDDEN, f32)
    for h in range(16):
        sl = slice(64 * h, 64 * h + 64)
        s = K[:, sl] @ q0[sl]
        p = np.exp(s - s.max())
        out0[sl] = (p @ V[:, sl]) / p.sum()
    out0 = out0.astype(BF16).astype(f32)
    return out0 @ np.asarray(Wo).astype(BF16).astype(f32).T


def kernel(x, Wq, Wk, Wv, Wo):
    from concourse import bass_utils

    x = np.asarray(x)
    B = x.shape[0]
    xb = x[0].astype(BF16)
    xt = np.ascontiguousarray(
        xb.reshape(8, 512, 8, 128).transpose(0, 3, 2, 1))

    def wprep(W, rs):
        wt = np.asarray(W)[rs, :].T.astype(BF16)
        return np.ascontiguousarray(
            wt.reshape(8, 128, OC).transpose(1, 0, 2))

    in_maps = []
    for d in range(N_CORES):
        rs = slice(OC * d, OC * (d + 1))
        in_maps.append({
            "xt": xt,
            "wqt": wprep(Wq, rs),
            "wkt": wprep(Wk, rs),
            "wvt": wprep(Wv, rs),
            "wot": np.ascontiguousarray(np.asarray(Wo)[:, rs].T.astype(BF16)),
        })

    if "nc" not in _CACHE:
        _CACHE["nc"] = _build()
    nc = _CACHE["nc"]

    res = bass_utils.run_bass_kernel_spmd(
        nc, in_maps, core_ids=list(range(N_CORES)),
        trace=bool(os.environ.get("KERNEL_TRACE")))
    global LAST_RESULTS
    LAST_RESULTS = res

    out = np.zeros((S, HIDDEN), np.float64)
    for r in res.results:
        out += r["partial"].astype(np.float64)
    out[0, :] = _host_row0(x, Wq, Wk, Wv, Wo)
    return out.reshape(B, S, HIDDEN).astype(np.float32)
